# revision 10
# baseline (speedup 1.0000x reference)
"""Trainium2 Bass kernel for nn_GAT_1675037246077 (2-layer GAT + linear head).

Strategy (8 NeuronCores, SPMD single NEFF), v2:
 - Destination-sharded: core c owns dst nodes [c*12544, (c+1)*12544); nodes padded
   to NPAD = 100352. Own nodes degree-sorted into 98 tiles of 128; edge grid per
   tile [128 dst x J_t] with J_t = cross-core max degree (equal-J groups, nt<=8).
 - Layer 1 (no gathers): host pre-expands x into a *stacked* layout packing 8
   edge-slots into the 128 partitions; one matmul vs blockdiag([Wa|I16]x8)
   yields per-slot [alpha_src(3) | x(16)] for 8 slots at once. Weighted x
   (p*x, 48 cols) is aggregated over slots via identity-matmul PSUM chains,
   divided by z, then W1 applied once per tile (blockdiag over heads) producing
   h1^T [96,128] directly; bias+selu; table2 row h2w = selu(...)@W2 via one more
   matmul per tile. Pad slots are zeroed by host masks (no correction terms).
 - Table2: 32 bf16 cols (h2w only) = 64B rows; alpha_src2 recovered on device as
   h2w . a_src2; alpha_dst2 kept core-local. AllGather in 2 row-chunks
   (renumbered table rows) to overlap with layer-1 tail.
 - Layer 2 gather: dma_gather (InstDMAGatherAnt) of 256B quads (4 rows) from the
   quad-view table [NPAD/4, 128], int16 quad indices, round-robin across 4 SWDGE
   queues (4x descriptor-generation parallelism). Sub-row selection and softmax
   weights fold into one mask-multiply; per-(slot,q) weighted rows reduce over q
   on DVE, aggregate over slots via identity matmuls, divide by z, selu, final
   head matmul per tile.
"""
import numpy as np
import ml_dtypes

from concourse import mybir, tile, bacc
import concourse.bass as bass
from concourse.bass_utils import run_bass_kernel_spmd
from concourse.masks import make_identity

P = 128
AF = mybir.ActivationFunctionType
ALU = mybir.AluOpType
AX = mybir.AxisListType
BF16 = mybir.dt.bfloat16
F32 = mybir.dt.float32
I16 = mybir.dt.int16
NPBF16 = ml_dtypes.bfloat16

SELU_SCALE = 1.0507009873554805
SELU_ALPHA_SCALE = 1.7580993408473766  # scale * alpha
NT_MAX = 8
NI_MAX = 1024  # max idxs per dma_gather instruction (HW limit)


class Cfg:
    def __init__(self, N, E, ncores, fin=16, h1=3, c1=32, c2=32, ncout=16):
        self.N, self.E, self.ncores = N, E, ncores
        self.FIN, self.H1, self.C1, self.C2, self.NCOUT = fin, h1, c1, c2, ncout
        self.CW1 = h1 * c1              # 96
        self.SH = ((N + ncores - 1) // ncores + P - 1) // P * P   # 12544
        self.TPC = self.SH // P                                    # 98
        self.NPAD = self.SH * ncores                               # 100352
        self.SH2 = self.SH // 2                                    # 6272
        self.T2 = self.TPC // 2                                    # 49 (chunk bnd)
        self.FV1 = h1 + h1 * fin        # 51: [p(3) | px(48)]
        self.SLOT1 = h1 + fin + h1 * fin  # not used directly
        self.TPC8 = (self.TPC + 7) // 8  # 13


SJ_CAP = 48


def _group_plan(J, t_break):
    """Runs of equal J, nt <= NT_MAX, nt*J <= SJ_CAP, forced break at t_break."""
    groups = []
    t = 0
    n = len(J)
    while t < n:
        j = J[t]
        assert j <= SJ_CAP, f"tile degree {j} exceeds SJ_CAP"
        nt = 1
        while (t + nt < n and J[t + nt] == j and nt < NT_MAX
               and (nt + 1) * j <= SJ_CAP and (t + nt) != t_break):
            nt += 1
        groups.append((t, nt, int(j)))
        t += nt
    return groups


def preprocess(cfg, x, edge_index, W1, a_src1, a_dst1, W2, a_src2, a_dst2):
    """Host-side sharding/packing. Returns (percore list, wpack dict, meta)."""
    N, E, NC = cfg.N, cfg.E, cfg.ncores
    SH, TPC, NPAD, SH2 = cfg.SH, cfg.TPC, cfg.NPAD, cfg.SH2
    FIN, H1, C1, C2, CW1 = cfg.FIN, cfg.H1, cfg.C1, cfg.C2, cfg.CW1

    loops = np.arange(N, dtype=np.int64)
    src = np.concatenate([edge_index[0].astype(np.int64), loops])
    dst = np.concatenate([edge_index[1].astype(np.int64), loops])

    deg = np.bincount(dst, minlength=NPAD)

    perms = []
    Jt_all = np.zeros((NC, TPC), np.int64)
    for c in range(NC):
        d = deg[c * SH:(c + 1) * SH]
        order = np.argsort(-d, kind="stable")
        perm = c * SH + order
        perms.append(perm)
        Jt_all[c] = d[order].reshape(TPC, P).max(1)
    J = np.maximum(Jt_all.max(0), 1)
    S = int(J.sum())

    # table-row numbering for 2-chunk AllGather:
    # slot < SH2: row = c*SH2 + slot ; else row = NPAD/2 + c*SH2 + (slot-SH2)
    pos2 = np.empty(NPAD, np.int64)
    for c in range(NC):
        slot = np.arange(SH)
        row = np.where(slot < SH2, c * SH2 + slot,
                       NPAD // 2 + c * SH2 + (slot - SH2))
        pos2[perms[c]] = row

    # CSR of edges by dst
    e_order = np.argsort(dst, kind="stable")
    src_sorted = src[e_order]
    starts = np.zeros(NPAD + 1, np.int64)
    np.cumsum(deg, out=starts[1:])

    offs = np.zeros(TPC + 1, np.int64)
    np.cumsum(J, out=offs[1:])

    groups = _group_plan(J, cfg.T2)
    # padded-slot stream for L1 stacking
    SJ8s = []
    for (t0, nt, Jg) in groups:
        SJ = nt * Jg
        SJ8s.append((SJ + 7) // 8 * 8)
    S8 = int(sum(SJ8s))
    NBLK = S8 // 8

    xpad = np.zeros((NPAD, FIN), np.float32)
    xpad[:N] = x
    xpadT_bf = np.ascontiguousarray(xpad.T).astype(NPBF16)

    percore = []
    for c in range(NC):
        perm = perms[c]
        idx1 = np.full((P, S), NPAD - 1, np.int64)   # src node per slot (pad->zero row)
        real = np.zeros((P, S), bool)
        for t in range(TPC):
            jt = int(J[t])
            o = int(offs[t])
            for p in range(P):
                node = perm[t * P + p]
                dg = int(deg[node])
                s0 = int(starts[node])
                idx1[p, o:o + dg] = src_sorted[s0:s0 + dg]
                real[p, o:o + dg] = True

        # --- L1 stacked x expansion ---
        idxpad = np.full((P, S8), NPAD - 1, np.int64)
        sp = 0
        for gi, (t0, nt, Jg) in enumerate(groups):
            SJ = nt * Jg
            o = int(offs[t0])
            idxpad[:, sp:sp + SJ] = idx1[:, o:o + SJ]
            sp += SJ8s[gi]
        G = xpadT_bf[:, idxpad.T.ravel()]                     # [16, S8*128]
        G = G.reshape(FIN, NBLK, 8, P).transpose(2, 0, 1, 3)  # [8,16,NBLK,128]
        xstk = np.ascontiguousarray(G.reshape(P, NBLK * P))

        # --- own-node stacked x (for alpha_dst) ---
        perm_pad = np.concatenate(
            [perm, np.full(cfg.TPC8 * 8 * P - SH, NPAD - 1, np.int64)])
        G2 = xpadT_bf[:, perm_pad].reshape(FIN, cfg.TPC8, 8, P)
        xown_stk = np.ascontiguousarray(
            G2.transpose(2, 0, 1, 3).reshape(P, cfg.TPC8 * P))

        # --- L2 quad indices + masks ---
        idx2 = pos2[idx1]                      # [P, S]
        idx2[~real] = 0
        qidx = (idx2 // 4).astype(np.int16)
        sub = (idx2 % 4).astype(np.int64)
        m4 = np.zeros((P, S, 4), NPBF16)
        m4[np.arange(P)[:, None], np.arange(S)[None, :], sub] = real.astype(NPBF16)
        m4 = np.ascontiguousarray(m4.reshape(P, S * 4))
        mask1 = np.ascontiguousarray(real.astype(NPBF16))

        # idx16 stream per group: i = s_local*128 + d -> [i%16, i//16]
        chunks = []
        for (t0, nt, Jg) in groups:
            SJ = nt * Jg
            o = int(offs[t0])
            stream = qidx[:, o:o + SJ].T.ravel()   # [SJ*128], i = s*128+d
            chunks.append(stream.reshape(-1, 16).T)  # [16, SJ*8]
        idx16 = np.concatenate(chunks, axis=1)       # [16, S*8]
        idx16 = np.ascontiguousarray(np.tile(idx16, (8, 1)))  # [128, S*8]

        percore.append(dict(own=perm.astype(np.int64), xstk=xstk,
                            xown_stk=xown_stk, idx16=idx16, m4=m4, mask1=mask1))

    # ---- weights ----
    wa1 = np.zeros((FIN, H1), np.float32)
    wad1 = np.zeros((FIN, H1), np.float32)
    for h in range(H1):
        wa1[:, h] = W1[:, h * C1:(h + 1) * C1] @ a_src1[h]
        wad1[:, h] = W1[:, h * C1:(h + 1) * C1] @ a_dst1[h]
    waug_blk1 = np.zeros((P, 8 * (H1 + FIN)), np.float32)   # [128, 152]
    wad_blk = np.zeros((P, 8 * H1), np.float32)             # [128, 24]
    for s in range(8):
        r = s * FIN
        cbase = s * (H1 + FIN)
        waug_blk1[r:r + FIN, cbase:cbase + H1] = wa1
        waug_blk1[r:r + FIN, cbase + H1:cbase + H1 + FIN] = np.eye(FIN)
        wad_blk[r:r + FIN, s * H1:(s + 1) * H1] = wad1
    w1blk = np.zeros((H1 * FIN, CW1), np.float32)           # [48, 96]
    for h in range(H1):
        w1blk[h * FIN:(h + 1) * FIN, h * C1:(h + 1) * C1] = \
            W1[:, h * C1:(h + 1) * C1]
    waug2 = np.zeros((CW1, C2 + 2), np.float32)             # [96, 34]
    waug2[:, 0] = W2 @ a_src2[0]
    waug2[:, 1:1 + C2] = W2
    waug2[:, 1 + C2] = W2 @ a_dst2[0]
    a2rep = np.broadcast_to(a_src2[0].astype(np.float32), (P, C2)).copy()

    wpack = dict(waug_blk1=waug_blk1, wad_blk=wad_blk, w1blk=w1blk,
                 waug2=waug2, a2rep=a2rep)
    meta = dict(J=[int(j) for j in J], offs=[int(o) for o in offs],
                S=S, S8=S8, NBLK=NBLK, groups=groups, SJ8s=SJ8s)
    return percore, wpack, meta


def build_nc(cfg, meta):
    J, offs, groups, SJ8s = meta["J"], meta["offs"], meta["groups"], meta["SJ8s"]
    S, S8, NBLK = meta["S"], meta["S8"], meta["NBLK"]
    TPC, NPAD, SH, SH2 = cfg.TPC, cfg.NPAD, cfg.SH, cfg.SH2
    FIN, H1, CW1, C2, NCOUT = cfg.FIN, cfg.H1, cfg.CW1, cfg.C2, cfg.NCOUT
    FV1 = cfg.FV1                     # 51
    SLOTW = H1 + FIN                  # 19
    NQ = NPAD // 4

    nc = bacc.Bacc("TRN2", target_bir_lowering=False,
                   num_devices=cfg.ncores, num_swdge_queues=4)

    # ---- I/O ----
    t_xstk = nc.dram_tensor("xstk", [P, NBLK * P], BF16, kind="ExternalInput")
    t_xown = nc.dram_tensor("xown_stk", [P, cfg.TPC8 * P], BF16, kind="ExternalInput")
    t_idx16 = nc.dram_tensor("idx16", [P, S * 8], I16, kind="ExternalInput")
    t_m4 = nc.dram_tensor("m4", [P, S * 4], BF16, kind="ExternalInput")
    t_mask1 = nc.dram_tensor("mask1", [P, S], BF16, kind="ExternalInput")
    t_wblk1 = nc.dram_tensor("waug_blk1", [P, 8 * SLOTW], BF16, kind="ExternalInput")
    t_wadb = nc.dram_tensor("wad_blk", [P, 8 * H1], BF16, kind="ExternalInput")
    t_w1blk = nc.dram_tensor("w1blk", [H1 * FIN, CW1], BF16, kind="ExternalInput")
    t_waug2 = nc.dram_tensor("waug2", [CW1, C2 + 2], BF16, kind="ExternalInput")
    t_wf = nc.dram_tensor("wf", [C2, NCOUT], BF16, kind="ExternalInput")
    t_a2rep = nc.dram_tensor("a2rep", [P, C2], BF16, kind="ExternalInput")
    t_b1T = nc.dram_tensor("b1T", [CW1, 1], F32, kind="ExternalInput")
    t_b2r = nc.dram_tensor("b2r", [P, C2], F32, kind="ExternalInput")
    t_bfr = nc.dram_tensor("bfr", [P, NCOUT], F32, kind="ExternalInput")
    t_out = nc.dram_tensor("out", [SH, NCOUT], F32, kind="ExternalOutput")

    t_cc_in = nc.dram_tensor("cc_in", [SH, C2], BF16)
    cc_space = "Shared" if cfg.ncores > 4 else "Local"
    t_cc_out = nc.dram_tensor("cc_out", [NQ, 4 * C2], BF16, addr_space=cc_space)

    qctr = [0]

    with tile.TileContext(nc) as tc:
        with (
            tc.tile_pool(name="res", bufs=1) as res,
            tc.tile_pool(name="pa", bufs=3) as pa,
            tc.tile_pool(name="pb", bufs=2) as pb,
            tc.tile_pool(name="fin", bufs=2) as fin,
            tc.tile_pool(name="psA", bufs=2, space="PSUM") as psA,
            tc.tile_pool(name="acc", bufs=2, space="PSUM") as accp,
            tc.tile_pool(name="aux", bufs=2, space="PSUM") as auxp,
            tc.tile_pool(name="hT", bufs=2, space="PSUM") as hTp,
        ):
            # ---- residents ----
            ident = res.tile([P, P], BF16)
            make_identity(nc, ident[:])
            wblk1 = res.tile([P, 8 * SLOTW], BF16)
            nc.sync.dma_start(wblk1[:], t_wblk1[:, :])
            wadb = res.tile([P, 8 * H1], BF16)
            nc.sync.dma_start(wadb[:], t_wadb[:, :])
            w1blk = res.tile([H1 * FIN, CW1], BF16)
            nc.sync.dma_start(w1blk[:], t_w1blk[:, :])
            waug2 = res.tile([CW1, C2 + 2], BF16)
            nc.sync.dma_start(waug2[:], t_waug2[:, :])
            wf = res.tile([C2, NCOUT], BF16)
            nc.sync.dma_start(wf[:], t_wf[:, :])
            a2rep = res.tile([P, C2], BF16)
            nc.sync.dma_start(a2rep[:], t_a2rep[:, :])
            b1T = res.tile([CW1, 1], F32)
            nc.sync.dma_start(b1T[:], t_b1T[:, :])
            b2r = res.tile([P, C2], F32)
            nc.sync.dma_start(b2r[:], t_b2r[:, :])
            bfr = res.tile([P, NCOUT], F32)
            nc.sync.dma_start(bfr[:], t_bfr[:, :])
            idx16 = res.tile([P, S * 8], I16)
            nc.sync.dma_start(idx16[:], t_idx16[:, :])
            m4 = res.tile([P, S * 4], BF16)
            nc.sync.dma_start(m4[:], t_m4[:, :])
            mask1 = res.tile([P, S], BF16)
            nc.sync.dma_start(mask1[:], t_mask1[:, :])
            xown = res.tile([P, cfg.TPC8 * P], BF16)
            nc.sync.dma_start(xown[:], t_xown[:, :])
            ad1 = res.tile([P, TPC * H1], F32)
            ad2 = res.tile([P, TPC], F32)

            # ---- alpha_dst1 for own nodes (stacked: 8 tiles per matmul) ----
            for I in range(cfg.TPC8):
                ps = psA.tile([P, 8 * SLOTW], F32, tag="ps_big")
                nc.tensor.matmul(ps[:, :8 * H1], lhsT=xown[:, I * P:(I + 1) * P],
                                 rhs=wadb[:], start=True, stop=True)
                ntt = min(8, TPC - I * 8)
                nc.vector.tensor_copy(ad1[:, I * 8 * H1:(I * 8 + ntt) * H1],
                                      ps[:, :ntt * H1])

            # ================= LAYER 1 + table2 build =================
            sp8 = 0    # padded slot offset (blocks)
            for gi, (t0, nt, Jg) in enumerate(groups):
                SJ = nt * Jg
                SJ8 = SJ8s[gi]
                nblk = SJ8 // 8
                o = offs[t0]

                xe = pa.tile([P, 6 * P], BF16, tag="xe")
                nc.sync.dma_start(xe[:, :nblk * P],
                                  t_xstk[:, sp8 * 16:(sp8 + nblk * 8) * 16])
                sc = pb.tile([P, 48 * SLOTW], BF16, tag="sc")
                for b in range(nblk):
                    ps = psA.tile([P, 8 * SLOTW], F32, tag="ps_big")
                    nc.tensor.matmul(ps[:], lhsT=xe[:, b * P:(b + 1) * P],
                                     rhs=wblk1[:], start=True, stop=True)
                    if b % 2 == 0:
                        nc.scalar.activation(sc[:, b * 8 * SLOTW:(b + 1) * 8 * SLOTW],
                                             ps[:], AF.Copy)
                    else:
                        nc.vector.tensor_copy(sc[:, b * 8 * SLOTW:(b + 1) * 8 * SLOTW],
                                              ps[:])

                # scores: s = alpha_s + ad1 ; p = exp(lrelu(s)) * mask1
                scv = sc[:, :SJ * SLOTW].rearrange("p (s w) -> p s w", w=SLOTW)
                s1 = pb.tile([P, 48 * H1], F32, tag="s1")
                s4 = s1[:, :SJ * H1].rearrange("p (t j h) -> p t j h", j=Jg, h=H1)
                adv = ad1[:].rearrange("p (t h) -> p t h", h=H1)[:, t0:t0 + nt, :]
                nc.vector.tensor_tensor(
                    out=s4,
                    in0=scv[:, :, 0:H1].rearrange("p (t j) h -> p t j h", j=Jg),
                    in1=adv[:, :, None, :].to_broadcast([P, nt, Jg, H1]),
                    op=ALU.add)
                r1 = pb.tile([P, 48 * H1], F32, tag="r1")
                nc.scalar.activation(r1[:, :SJ * H1], s1[:, :SJ * H1], AF.Relu)
                nc.vector.tensor_scalar(out=r1[:, :SJ * H1], in0=r1[:, :SJ * H1],
                                        scalar1=4.0, scalar2=None, op0=ALU.mult)
                nc.vector.tensor_tensor(out=s1[:, :SJ * H1], in0=s1[:, :SJ * H1],
                                        in1=r1[:, :SJ * H1], op=ALU.add)
                pt = pb.tile([P, 48 * H1], F32, tag="pt")
                nc.scalar.activation(pt[:, :SJ * H1], s1[:, :SJ * H1], AF.Exp,
                                     scale=0.2)
                rhs2 = pb.tile([P, 48 * FV1], BF16, tag="rhs2")
                r2 = rhs2[:, :SJ * FV1].rearrange("p (s f) -> p s f", f=FV1)
                nc.vector.tensor_tensor(
                    out=r2[:, :, 0:H1],
                    in0=pt[:, :SJ * H1].rearrange("p (s h) -> p s h", h=H1),
                    in1=mask1[:, o:o + SJ][:, :, None].to_broadcast([P, SJ, H1]),
                    op=ALU.mult)
                # px: rhs2[:, s, 3+h*16+f] = x[s,f] * p[s,h]
                nc.vector.tensor_tensor(
                    out=r2[:, :, H1:].rearrange("p s (h f) -> p s h f", h=H1),
                    in0=scv[:, :, None, H1:].to_broadcast([P, SJ, H1, FIN]),
                    in1=r2[:, :, 0:H1].to_broadcast([P, SJ, H1, FIN]),
                    op=ALU.mult)

                # aggregate over j (PSUM accumulation)
                acc = accp.tile([P, NT_MAX * FV1], F32, tag="agg")
                for j in range(Jg):
                    nc.tensor.matmul(
                        acc[:, :nt * FV1], lhsT=ident[:],
                        rhs=rhs2[:, :SJ * FV1].rearrange(
                            "p (t j f) -> p t (j f)", j=Jg,
                            f=FV1)[:, :, j * FV1:(j + 1) * FV1],
                        start=(j == 0), stop=(j == Jg - 1))

                # z, reciprocal, aggx/z (bf16)
                av = acc[:, :nt * FV1].rearrange("p (t f) -> p t f", f=FV1)
                zr = fin.tile([P, NT_MAX * H1], F32, tag="zr")
                nc.vector.tensor_scalar(
                    out=zr[:, :nt * H1].rearrange("p (t h) -> p t h", h=H1),
                    in0=av[:, :, 0:H1], scalar1=1e-16, scalar2=None, op0=ALU.add)
                nc.vector.reciprocal(zr[:, :nt * H1], zr[:, :nt * H1])
                axz = fin.tile([P, NT_MAX * H1 * FIN], BF16, tag="axz")
                nc.vector.tensor_tensor(
                    out=axz[:, :nt * H1 * FIN].rearrange(
                        "p (t h f) -> p t h f", h=H1, f=FIN),
                    in0=av[:, :, H1:].rearrange("p t (h f) -> p t h f", h=H1),
                    in1=zr[:, :nt * H1].rearrange(
                        "p (t h) -> p t h", h=H1).to_broadcast([P, nt, H1, FIN]),
                    op=ALU.mult)

                # per tile: transpose -> blockdiag W1 -> h1T [96,128]
                hseg = fin.tile([CW1, NT_MAX * P], F32, tag="hseg")
                for i in range(nt):
                    tp = auxp.tile([H1 * FIN, P], BF16, tag="tp")
                    nc.tensor.transpose(tp[:], axz[:, i * H1 * FIN:(i + 1) * H1 * FIN],
                                        ident[:])
                    axzT = pa.tile([H1 * FIN, P], BF16, tag="axzT")
                    nc.scalar.activation(axzT[:], tp[:], AF.Copy)
                    h1T = hTp.tile([CW1, P], F32, tag="h1T")
                    nc.tensor.matmul(h1T[:], lhsT=w1blk[:], rhs=axzT[:],
                                     start=True, stop=True)
                    nc.vector.tensor_tensor(
                        out=hseg[:, i * P:(i + 1) * P], in0=h1T[:],
                        in1=b1T[:, 0:1].to_broadcast([CW1, P]), op=ALU.add)
                # selu on [96, nt*128] -> h2T bf16
                h2T = pa.tile([CW1, NT_MAX * P], BF16, tag="h2T")
                rr = fin.tile([CW1, NT_MAX * P], F32, tag="rrT")
                nc.scalar.activation(rr[:, :nt * P], hseg[:, :nt * P], AF.Relu)
                ww = fin.tile([CW1, NT_MAX * P], F32, tag="wwT")
                nc.vector.tensor_tensor(out=ww[:, :nt * P], in0=hseg[:, :nt * P],
                                        in1=rr[:, :nt * P], op=ALU.subtract)
                nc.scalar.activation(ww[:, :nt * P], ww[:, :nt * P], AF.Exp)
                nc.vector.tensor_scalar(out=ww[:, :nt * P], in0=ww[:, :nt * P],
                                        scalar1=SELU_ALPHA_SCALE,
                                        scalar2=-SELU_ALPHA_SCALE,
                                        op0=ALU.mult, op1=ALU.add)
                nc.vector.tensor_scalar(out=rr[:, :nt * P], in0=rr[:, :nt * P],
                                        scalar1=SELU_SCALE, scalar2=None,
                                        op0=ALU.mult)
                nc.vector.tensor_tensor(out=h2T[:, :nt * P], in0=ww[:, :nt * P],
                                        in1=rr[:, :nt * P], op=ALU.add)

                # per tile: table2 row [as2|h2w|ad2] = h2T^T @ waug2
                st2 = fin.tile([P, NT_MAX * C2], BF16, tag="st2")
                for i in range(nt):
                    t = t0 + i
                    t2 = psA.tile([P, 8 * SLOTW], F32, tag="ps_big")
                    nc.tensor.matmul(t2[:, :C2 + 2], lhsT=h2T[:, i * P:(i + 1) * P],
                                     rhs=waug2[:], start=True, stop=True)
                    nc.vector.tensor_copy(ad2[:, t:t + 1], t2[:, C2 + 1:C2 + 2])
                    nc.scalar.activation(st2[:, i * C2:(i + 1) * C2],
                                         t2[:, 1:1 + C2], AF.Copy)
                dst_ap = t_cc_in[t0 * P:(t0 + nt) * P, :].rearrange(
                    "(i p) c -> p i c", p=P)
                nc.sync.dma_start(dst_ap, st2[:, :nt * C2].rearrange(
                    "p (i c) -> p i c", c=C2))
                sp8 += nblk * 8

                # chunked AllGather as soon as each half of the tiles is done
                if t0 + nt == cfg.T2:
                    nc.gpsimd.collective_compute(
                        "AllGather", ALU.bypass,
                        replica_groups=[list(range(cfg.ncores))],
                        ins=[t_cc_in[0:SH2, :].opt()],
                        outs=[t_cc_out[0:NQ // 2, :].opt()],
                    )
            nc.gpsimd.collective_compute(
                "AllGather", ALU.bypass,
                replica_groups=[list(range(cfg.ncores))],
                ins=[t_cc_in[SH2:SH, :].opt()],
                outs=[t_cc_out[NQ // 2:NQ, :].opt()],
            )

            # ================= LAYER 2 + head =================
            FV2 = 1 + C2  # 33
            for gi, (t0, nt, Jg) in enumerate(groups):
                SJ = nt * Jg
                o = offs[t0]

                gath = pb.tile([P, 48 * 4 * C2], BF16, tag="gath")
                nidx = SJ * P
                base16 = o * 8
                cpos = 0
                while cpos < nidx:
                    ni = min(NI_MAX, nidx - cpos)
                    nc.gpsimd.dma_gather(
                        out_ap=gath[:, cpos:cpos + ni].rearrange(
                            "p (s e) -> p s e", e=4 * C2),
                        in_ap=t_cc_out[:, :],
                        idxs_ap=idx16[:, base16 + cpos // 16:base16 + (cpos + ni) // 16],
                        num_idxs=ni, num_idxs_reg=ni, elem_size=4 * C2,
                        queue_num=qctr[0] % 4)
                    qctr[0] += 1
                    cpos += ni

                # alpha_src2 per (slot,q): dots = (g . a2), masked sum over q
                gm = pb.tile([P, 48 * 4 * C2], BF16, tag="gm")
                nc.vector.tensor_tensor(
                    out=gm[:, :SJ * 4 * C2].rearrange("p (u c) -> p u c", c=C2),
                    in0=gath[:, :SJ * 4 * C2].rearrange("p (u c) -> p u c", c=C2),
                    in1=a2rep[:][:, None, :].to_broadcast([P, SJ * 4, C2]),
                    op=ALU.mult)
                dots = pb.tile([P, 48 * 4], F32, tag="dots")
                nc.vector.tensor_reduce(
                    out=dots[:, :SJ * 4],
                    in_=gm[:, :SJ * 4 * C2].rearrange("p (u c) -> p u c", c=C2),
                    axis=AX.X, op=ALU.add)
                nc.vector.tensor_tensor(
                    out=dots[:, :SJ * 4], in0=dots[:, :SJ * 4],
                    in1=m4[:, o * 4:(o + SJ) * 4], op=ALU.mult)
                as2 = pb.tile([P, 48], F32, tag="as2")
                nc.vector.tensor_reduce(
                    out=as2[:, :SJ],
                    in_=dots[:, :SJ * 4].rearrange("p (s q) -> p s q", q=4),
                    axis=AX.X, op=ALU.add)
                # p2 = exp(lrelu(as2 + ad2))
                nc.vector.tensor_tensor(
                    out=as2[:, :SJ].rearrange("p (t j) -> p t j", j=Jg),
                    in0=as2[:, :SJ].rearrange("p (t j) -> p t j", j=Jg),
                    in1=ad2[:, t0:t0 + nt][:, :, None].to_broadcast([P, nt, Jg]),
                    op=ALU.add)
                r2t = pb.tile([P, 48], F32, tag="r2t")
                nc.scalar.activation(r2t[:, :SJ], as2[:, :SJ], AF.Relu)
                nc.vector.tensor_scalar(out=r2t[:, :SJ], in0=r2t[:, :SJ],
                                        scalar1=4.0, scalar2=None, op0=ALU.mult)
                nc.vector.tensor_tensor(out=as2[:, :SJ], in0=as2[:, :SJ],
                                        in1=r2t[:, :SJ], op=ALU.add)
                p2 = pb.tile([P, 48], BF16, tag="p2")
                nc.scalar.activation(p2[:, :SJ], as2[:, :SJ], AF.Exp, scale=0.2)
                # pm = p2 (bcast q) * m4
                pm = pb.tile([P, 48 * 4], BF16, tag="pm")
                nc.vector.tensor_tensor(
                    out=pm[:, :SJ * 4].rearrange("p (s q) -> p s q", q=4),
                    in0=p2[:, :SJ][:, :, None].to_broadcast([P, SJ, 4]),
                    in1=m4[:, o * 4:(o + SJ) * 4].rearrange("p (s q) -> p s q", q=4),
                    op=ALU.mult)
                # weighted rows: tmp = gath * pm (bcast c), sum over q (4 blocks)
                nc.vector.tensor_tensor(
                    out=gm[:, :SJ * 4 * C2].rearrange("p (u c) -> p u c", c=C2),
                    in0=gath[:, :SJ * 4 * C2].rearrange("p (u c) -> p u c", c=C2),
                    in1=pm[:, :SJ * 4][:, :, None].to_broadcast([P, SJ * 4, C2]),
                    op=ALU.mult)
                gm4 = gm[:, :SJ * 4 * C2].rearrange("p (s u) -> p s u", u=4 * C2)
                tv = pb.tile([P, 48 * FV2], F32, tag="tv")
                tvv = tv[:, :SJ * FV2].rearrange("p (s f) -> p s f", f=FV2)
                nc.vector.tensor_reduce(
                    out=tvv[:, :, 0:1],
                    in_=pm[:, :SJ * 4].rearrange("p (s q) -> p s q", q=4),
                    axis=AX.X, op=ALU.add)
                ta = pb.tile([P, 48 * C2], F32, tag="ta")
                tav = ta[:, :SJ * C2].rearrange("p (s c) -> p s c", c=C2)
                nc.vector.tensor_tensor(out=tav, in0=gm4[:, :, 0:C2],
                                        in1=gm4[:, :, C2:2 * C2], op=ALU.add)
                nc.vector.tensor_tensor(
                    out=tvv[:, :, 1:], in0=gm4[:, :, 2 * C2:3 * C2],
                    in1=gm4[:, :, 3 * C2:4 * C2], op=ALU.add)
                nc.vector.tensor_tensor(out=tvv[:, :, 1:], in0=tvv[:, :, 1:],
                                        in1=tav, op=ALU.add)
                rhs2b = pb.tile([P, 48 * FV2], BF16, tag="rhs2b")
                nc.scalar.activation(rhs2b[:, :SJ * FV2], tv[:, :SJ * FV2], AF.Copy)

                acc = accp.tile([P, NT_MAX * FV1], F32, tag="agg")
                for j in range(Jg):
                    nc.tensor.matmul(
                        acc[:, :nt * FV2], lhsT=ident[:],
                        rhs=rhs2b[:, :SJ * FV2].rearrange(
                            "p (t j f) -> p t (j f)", j=Jg,
                            f=FV2)[:, :, j * FV2:(j + 1) * FV2],
                        start=(j == 0), stop=(j == Jg - 1))

                av = acc[:, :nt * FV2].rearrange("p (t f) -> p t f", f=FV2)
                zr2 = fin.tile([P, NT_MAX], F32, tag="zr2")
                nc.vector.tensor_scalar(
                    out=zr2[:, :nt][:, :, None],
                    in0=av[:, :, 0:1], scalar1=1e-16, scalar2=None, op0=ALU.add)
                nc.vector.reciprocal(zr2[:, :nt], zr2[:, :nt])
                h3f = fin.tile([P, NT_MAX * C2], F32, tag="h3f")
                h3v = h3f[:, :nt * C2].rearrange("p (t c) -> p t c", c=C2)
                nc.vector.tensor_tensor(
                    out=h3v, in0=av[:, :, 1:],
                    in1=zr2[:, :nt][:, :, None].to_broadcast([P, nt, C2]),
                    op=ALU.mult)
                nc.vector.tensor_tensor(
                    out=h3v, in0=h3v,
                    in1=b2r[:][:, None, :].to_broadcast([P, nt, C2]), op=ALU.add)
                # selu -> h3 bf16
                rr3 = fin.tile([P, NT_MAX * C2], F32, tag="rr3")
                nc.scalar.activation(rr3[:, :nt * C2], h3f[:, :nt * C2], AF.Relu)
                ww3 = fin.tile([P, NT_MAX * C2], F32, tag="ww3")
                nc.vector.tensor_tensor(out=ww3[:, :nt * C2], in0=h3f[:, :nt * C2],
                                        in1=rr3[:, :nt * C2], op=ALU.subtract)
                nc.scalar.activation(ww3[:, :nt * C2], ww3[:, :nt * C2], AF.Exp)
                nc.vector.tensor_scalar(out=ww3[:, :nt * C2], in0=ww3[:, :nt * C2],
                                        scalar1=SELU_ALPHA_SCALE,
                                        scalar2=-SELU_ALPHA_SCALE,
                                        op0=ALU.mult, op1=ALU.add)
                nc.vector.tensor_scalar(out=rr3[:, :nt * C2], in0=rr3[:, :nt * C2],
                                        scalar1=SELU_SCALE, scalar2=None,
                                        op0=ALU.mult)
                h3 = fin.tile([P, NT_MAX * C2], BF16, tag="h3")
                nc.vector.tensor_tensor(out=h3[:, :nt * C2], in0=ww3[:, :nt * C2],
                                        in1=rr3[:, :nt * C2], op=ALU.add)

                # final head per tile
                ost = fin.tile([P, NT_MAX * NCOUT], F32, tag="ost")
                for i in range(nt):
                    tp = auxp.tile([H1 * FIN, P], BF16, tag="tp")
                    nc.tensor.transpose(tp[:C2, :], h3[:, i * C2:(i + 1) * C2],
                                        ident[:])
                    h3T = pa.tile([C2, P], BF16, tag="h3T")
                    nc.scalar.activation(h3T[:], tp[:C2, :], AF.Copy)
                    pso = psA.tile([P, 8 * SLOTW], F32, tag="ps_big")
                    nc.tensor.matmul(pso[:, :NCOUT], lhsT=h3T[:], rhs=wf[:],
                                     start=True, stop=True)
                    nc.vector.tensor_tensor(out=ost[:, i * NCOUT:(i + 1) * NCOUT],
                                            in0=pso[:, :NCOUT], in1=bfr[:],
                                            op=ALU.add)
                dst_ap = t_out[t0 * P:(t0 + nt) * P, :].rearrange(
                    "(i p) c -> p i c", p=P)
                nc.sync.dma_start(dst_ap, ost[:, :nt * NCOUT].rearrange(
                    "p (i c) -> p i c", c=NCOUT))

    nc.compile()
    return nc


def _make_inputs(cfg, percore, wpack, inputs):
    b1 = np.asarray(inputs["b1"], np.float32)
    b2 = np.asarray(inputs["b2"], np.float32)
    bf = np.asarray(inputs["bf"], np.float32)
    wf = np.asarray(inputs["Wf"], np.float32).astype(NPBF16)
    b1T = np.ascontiguousarray(b1[:, None])
    b2r = np.broadcast_to(b2, (P, cfg.C2)).copy()
    bfr = np.broadcast_to(bf, (P, cfg.NCOUT)).copy()
    shared = {
        "waug_blk1": wpack["waug_blk1"].astype(NPBF16),
        "wad_blk": wpack["wad_blk"].astype(NPBF16),
        "w1blk": wpack["w1blk"].astype(NPBF16),
        "waug2": wpack["waug2"].astype(NPBF16),
        "a2rep": wpack["a2rep"].astype(NPBF16),
        "wf": wf, "b1T": b1T, "b2r": b2r, "bfr": bfr,
    }
    in_maps = []
    for c in range(cfg.ncores):
        pc = percore[c]
        m = dict(shared)
        m.update({"xstk": pc["xstk"], "xown_stk": pc["xown_stk"],
                  "idx16": pc["idx16"], "m4": pc["m4"], "mask1": pc["mask1"]})
        in_maps.append(m)
    return in_maps


def _assemble(cfg, percore, results):
    out = np.zeros((cfg.NPAD, cfg.NCOUT), np.float32)
    for c in range(cfg.ncores):
        out[percore[c]["own"]] = results[c]["out"]
    return out[:cfg.N]


def kernel(**inputs) -> np.ndarray:
    cfg = Cfg(N=100000, E=800000, ncores=8)
    percore, wpack, meta = preprocess(
        cfg,
        np.asarray(inputs["x"], np.float32),
        np.asarray(inputs["edge_index"]),
        np.asarray(inputs["W1"], np.float32),
        np.asarray(inputs["a_src1"], np.float32),
        np.asarray(inputs["a_dst1"], np.float32),
        np.asarray(inputs["W2"], np.float32),
        np.asarray(inputs["a_src2"], np.float32),
        np.asarray(inputs["a_dst2"], np.float32),
    )
    nc = build_nc(cfg, meta)
    in_maps = _make_inputs(cfg, percore, wpack, inputs)
    res = run_bass_kernel_spmd(nc, in_maps, core_ids=list(range(cfg.ncores)))
    return _assemble(cfg, percore, res.results)


if __name__ == "__main__":
    import reference as R
    inputs = R.setup_inputs()
    out = kernel(**{k: np.asarray(v) for k, v in inputs.items()})
    print("out", out.shape, out.dtype)


# revision 21
# speedup vs baseline: 1.4122x; 1.4122x over previous
"""Trainium2 Bass kernel for nn_GAT_1675037246077 (2-layer GAT + linear head).

Strategy (8 NeuronCores, SPMD single NEFF), v2:
 - Destination-sharded: core c owns dst nodes [c*12544, (c+1)*12544); nodes padded
   to NPAD = 100352. Own nodes degree-sorted into 98 tiles of 128; edge grid per
   tile [128 dst x J_t] with J_t = cross-core max degree (equal-J groups, nt<=8).
 - Layer 1 (no gathers): host pre-expands x into a *stacked* layout packing 8
   edge-slots into the 128 partitions; one matmul vs blockdiag([Wa|I16]x8)
   yields per-slot [alpha_src(3) | x(16)] for 8 slots at once. Weighted x
   (p*x, 48 cols) is aggregated over slots via identity-matmul PSUM chains,
   divided by z, then W1 applied once per tile (blockdiag over heads) producing
   h1^T [96,128] directly; bias+selu; table2 row h2w = selu(...)@W2 via one more
   matmul per tile. Pad slots are zeroed by host masks (no correction terms).
 - Table2: 32 bf16 cols (h2w only) = 64B rows; alpha_src2 recovered on device as
   h2w . a_src2; alpha_dst2 kept core-local. AllGather in 2 row-chunks
   (renumbered table rows) to overlap with layer-1 tail.
 - Layer 2 gather: dma_gather (InstDMAGatherAnt) of 256B quads (4 rows) from the
   quad-view table [NPAD/4, 128], int16 quad indices, round-robin across 4 SWDGE
   queues (4x descriptor-generation parallelism). Sub-row selection and softmax
   weights fold into one mask-multiply; per-(slot,q) weighted rows reduce over q
   on DVE, aggregate over slots via identity matmuls, divide by z, selu, final
   head matmul per tile.
"""
import numpy as np
import ml_dtypes

from concourse import mybir, tile, bacc
import concourse.bass as bass
from concourse.bass_utils import run_bass_kernel_spmd
from concourse.masks import make_identity

P = 128
AF = mybir.ActivationFunctionType
ALU = mybir.AluOpType
AX = mybir.AxisListType
BF16 = mybir.dt.bfloat16
F32 = mybir.dt.float32
I16 = mybir.dt.int16
NPBF16 = ml_dtypes.bfloat16

SELU_SCALE = 1.0507009873554805
SELU_ALPHA_SCALE = 1.7580993408473766  # scale * alpha
NT1, CAP1 = 10, 90    # layer-1 groups (PSUM: 10*51*4 = 2040B <= bank)
NT2, CAP2 = 8, 48     # layer-2 groups (gather tiles sized for SJ<=48)
NI_MAX = 1024         # max idxs per dma_gather instruction (HW limit)


class Cfg:
    def __init__(self, N, E, ncores, fin=16, h1=3, c1=32, c2=32, ncout=16):
        self.N, self.E, self.ncores = N, E, ncores
        self.FIN, self.H1, self.C1, self.C2, self.NCOUT = fin, h1, c1, c2, ncout
        self.CW1 = h1 * c1              # 96
        self.SH = ((N + ncores - 1) // ncores + P - 1) // P * P   # 12544
        self.TPC = self.SH // P                                    # 98
        self.NPAD = self.SH * ncores                               # 100352
        self.SH2 = self.SH // 2                                    # 6272
        self.T2 = self.TPC // 2                                    # 49 (chunk bnd)
        self.FV1 = h1 + h1 * fin        # 51: [p(3) | px(48)]
        self.SLOT1 = h1 + fin + h1 * fin  # not used directly
        self.TPC8 = (self.TPC + 7) // 8  # 13


def _group_plan(J, t_break, nt_max, sj_cap):
    """Runs of equal J, nt <= nt_max, nt*J <= sj_cap, forced break at t_break."""
    groups = []
    t = 0
    n = len(J)
    while t < n:
        j = J[t]
        assert j <= sj_cap, f"tile degree {j} exceeds cap {sj_cap}"
        nt = 1
        while (t + nt < n and J[t + nt] == j and nt < nt_max
               and (nt + 1) * j <= sj_cap and (t + nt) != t_break):
            nt += 1
        groups.append((t, nt, int(j)))
        t += nt
    return groups


def preprocess(cfg, x, edge_index, W1, a_src1, a_dst1, W2, a_src2, a_dst2):
    """Host-side sharding/packing. Returns (percore list, wpack dict, meta)."""
    N, E, NC = cfg.N, cfg.E, cfg.ncores
    SH, TPC, NPAD, SH2 = cfg.SH, cfg.TPC, cfg.NPAD, cfg.SH2
    FIN, H1, C1, C2, CW1 = cfg.FIN, cfg.H1, cfg.C1, cfg.C2, cfg.CW1

    loops = np.arange(N, dtype=np.int64)
    src = np.concatenate([edge_index[0].astype(np.int64), loops])
    dst = np.concatenate([edge_index[1].astype(np.int64), loops])

    deg = np.bincount(dst, minlength=NPAD)

    perms = []
    Jt_all = np.zeros((NC, TPC), np.int64)
    for c in range(NC):
        d = deg[c * SH:(c + 1) * SH]
        order = np.argsort(-d, kind="stable")
        perm = c * SH + order
        perms.append(perm)
        Jt_all[c] = d[order].reshape(TPC, P).max(1)
    J = np.maximum(Jt_all.max(0), 1)
    S = int(J.sum())

    # table-row numbering for 2-chunk AllGather:
    # slot < SH2: row = c*SH2 + slot ; else row = NPAD/2 + c*SH2 + (slot-SH2)
    pos2 = np.empty(NPAD, np.int64)
    for c in range(NC):
        slot = np.arange(SH)
        row = np.where(slot < SH2, c * SH2 + slot,
                       NPAD // 2 + c * SH2 + (slot - SH2))
        pos2[perms[c]] = row

    # CSR of edges by dst
    e_order = np.argsort(dst, kind="stable")
    src_sorted = src[e_order]
    starts = np.zeros(NPAD + 1, np.int64)
    np.cumsum(deg, out=starts[1:])

    offs = np.zeros(TPC + 1, np.int64)
    np.cumsum(J, out=offs[1:])

    groups1 = _group_plan(J, cfg.T2, NT1, CAP1)
    groups2 = _group_plan(J, cfg.T2, NT2, CAP2)
    # padded-slot stream for L1 stacking
    SJ8s = []
    for (t0, nt, Jg) in groups1:
        SJ = nt * Jg
        SJ8s.append((SJ + 7) // 8 * 8)
    S8 = int(sum(SJ8s))
    NBLK = S8 // 8

    xpad = np.zeros((NPAD, FIN), np.float32)
    xpad[:N] = x
    xpadT_bf = np.ascontiguousarray(xpad.T).astype(NPBF16)

    percore = []
    for c in range(NC):
        perm = perms[c]
        idx1 = np.full((P, S), NPAD - 1, np.int64)   # src node per slot (pad->zero row)
        real = np.zeros((P, S), bool)
        for t in range(TPC):
            jt = int(J[t])
            o = int(offs[t])
            for p in range(P):
                node = perm[t * P + p]
                dg = int(deg[node])
                s0 = int(starts[node])
                idx1[p, o:o + dg] = src_sorted[s0:s0 + dg]
                real[p, o:o + dg] = True

        # --- L1 stacked x expansion ---
        idxpad = np.full((P, S8), NPAD - 1, np.int64)
        sp = 0
        for gi, (t0, nt, Jg) in enumerate(groups1):
            SJ = nt * Jg
            o = int(offs[t0])
            idxpad[:, sp:sp + SJ] = idx1[:, o:o + SJ]
            sp += SJ8s[gi]
        G = xpadT_bf[:, idxpad.T.ravel()]                     # [16, S8*128]
        G = G.reshape(FIN, NBLK, 8, P).transpose(2, 0, 1, 3)  # [8,16,NBLK,128]
        xstk = np.ascontiguousarray(G.reshape(P, NBLK * P))

        # --- own-node stacked x (for alpha_dst) ---
        perm_pad = np.concatenate(
            [perm, np.full(cfg.TPC8 * 8 * P - SH, NPAD - 1, np.int64)])
        G2 = xpadT_bf[:, perm_pad].reshape(FIN, cfg.TPC8, 8, P)
        xown_stk = np.ascontiguousarray(
            G2.transpose(2, 0, 1, 3).reshape(P, cfg.TPC8 * P))

        # --- L2 quad indices + masks ---
        idx2 = pos2[idx1]                      # [P, S]
        idx2[~real] = 0
        qidx = (idx2 // 4).astype(np.int16)
        sub = (idx2 % 4).astype(np.int64)
        m4 = np.zeros((P, S, 4), NPBF16)
        m4[np.arange(P)[:, None], np.arange(S)[None, :], sub] = real.astype(NPBF16)
        m4 = np.ascontiguousarray(m4.reshape(P, S * 4))
        mask1 = np.ascontiguousarray(real.astype(NPBF16))

        # idx16 stream per L2 group: i = s_local*128 + d -> [i%16, i//16]
        chunks = []
        for (t0, nt, Jg) in groups2:
            SJ = nt * Jg
            o = int(offs[t0])
            stream = qidx[:, o:o + SJ].T.ravel()   # [SJ*128], i = s*128+d
            chunks.append(stream.reshape(-1, 16).T)  # [16, SJ*8]
        idx16 = np.concatenate(chunks, axis=1)       # [16, S*8]
        idx16 = np.ascontiguousarray(np.tile(idx16, (8, 1)))  # [128, S*8]

        percore.append(dict(own=perm.astype(np.int64), xstk=xstk,
                            xown_stk=xown_stk, idx16=idx16, m4=m4, mask1=mask1))

    # ---- weights ----
    wa1 = np.zeros((FIN, H1), np.float32)
    wad1 = np.zeros((FIN, H1), np.float32)
    for h in range(H1):
        wa1[:, h] = W1[:, h * C1:(h + 1) * C1] @ a_src1[h]
        wad1[:, h] = W1[:, h * C1:(h + 1) * C1] @ a_dst1[h]
    waug_blk1 = np.zeros((P, 8 * (H1 + FIN)), np.float32)   # [128, 152]
    wad_blk = np.zeros((P, 8 * H1), np.float32)             # [128, 24]
    for s in range(8):
        r = s * FIN
        cbase = s * (H1 + FIN)
        waug_blk1[r:r + FIN, cbase:cbase + H1] = wa1
        waug_blk1[r:r + FIN, cbase + H1:cbase + H1 + FIN] = np.eye(FIN)
        wad_blk[r:r + FIN, s * H1:(s + 1) * H1] = wad1
    w1blk = np.zeros((H1 * FIN, CW1), np.float32)           # [48, 96]
    for h in range(H1):
        w1blk[h * FIN:(h + 1) * FIN, h * C1:(h + 1) * C1] = \
            W1[:, h * C1:(h + 1) * C1]
    waug2 = np.zeros((CW1, C2 + 2), np.float32)             # [96, 34]
    waug2[:, 0] = W2 @ a_src2[0]
    waug2[:, 1:1 + C2] = W2
    waug2[:, 1 + C2] = W2 @ a_dst2[0]
    a2rep = np.broadcast_to(a_src2[0].astype(np.float32), (P, C2)).copy()

    wpack = dict(waug_blk1=waug_blk1, wad_blk=wad_blk, w1blk=w1blk,
                 waug2=waug2, a2rep=a2rep)
    meta = dict(J=[int(j) for j in J], offs=[int(o) for o in offs],
                S=S, S8=S8, NBLK=NBLK, groups=groups1, groups1=groups1,
                groups2=groups2, SJ8s=SJ8s)
    return percore, wpack, meta


def build_nc(cfg, meta):
    J, offs, SJ8s = meta["J"], meta["offs"], meta["SJ8s"]
    groups1, groups2 = meta["groups1"], meta["groups2"]
    S, S8, NBLK = meta["S"], meta["S8"], meta["NBLK"]
    TPC, NPAD, SH, SH2 = cfg.TPC, cfg.NPAD, cfg.SH, cfg.SH2
    FIN, H1, CW1, C2, NCOUT = cfg.FIN, cfg.H1, cfg.CW1, cfg.C2, cfg.NCOUT
    FV1 = cfg.FV1                     # 51
    SLOTW = H1 + FIN                  # 19
    NQ = NPAD // 4

    nc = bacc.Bacc("TRN2", target_bir_lowering=False,
                   num_devices=cfg.ncores, num_swdge_queues=4)

    # ---- I/O ----
    t_xstk = nc.dram_tensor("xstk", [P, NBLK * P], BF16, kind="ExternalInput")
    t_xown = nc.dram_tensor("xown_stk", [P, cfg.TPC8 * P], BF16, kind="ExternalInput")
    t_idx16 = nc.dram_tensor("idx16", [P, S * 8], I16, kind="ExternalInput")
    t_m4 = nc.dram_tensor("m4", [P, S * 4], BF16, kind="ExternalInput")
    t_mask1 = nc.dram_tensor("mask1", [P, S], BF16, kind="ExternalInput")
    t_wblk1 = nc.dram_tensor("waug_blk1", [P, 8 * SLOTW], BF16, kind="ExternalInput")
    t_wadb = nc.dram_tensor("wad_blk", [P, 8 * H1], BF16, kind="ExternalInput")
    t_w1blk = nc.dram_tensor("w1blk", [H1 * FIN, CW1], BF16, kind="ExternalInput")
    t_waug2 = nc.dram_tensor("waug2", [CW1, C2 + 2], BF16, kind="ExternalInput")
    t_wf = nc.dram_tensor("wf", [C2, NCOUT], BF16, kind="ExternalInput")
    t_a2rep = nc.dram_tensor("a2rep", [P, C2], BF16, kind="ExternalInput")
    t_b1T = nc.dram_tensor("b1T", [CW1, 1], F32, kind="ExternalInput")
    t_b2r = nc.dram_tensor("b2r", [P, C2], F32, kind="ExternalInput")
    t_bfr = nc.dram_tensor("bfr", [P, NCOUT], F32, kind="ExternalInput")
    t_out = nc.dram_tensor("out", [SH, NCOUT], F32, kind="ExternalOutput")

    t_cc_in = nc.dram_tensor("cc_in", [SH, C2], BF16)
    cc_space = "Shared" if cfg.ncores > 4 else "Local"
    t_cc_out = nc.dram_tensor("cc_out", [NQ, 4 * C2], BF16, addr_space=cc_space)

    qctr = [0]

    with tile.TileContext(nc) as tc:
        with (
            tc.tile_pool(name="res", bufs=1) as res,
            tc.tile_pool(name="pa", bufs=3) as pa,
            tc.tile_pool(name="pb", bufs=2) as pb,
            tc.tile_pool(name="pg", bufs=3) as pg,
            tc.tile_pool(name="fin", bufs=2) as fin,
            tc.tile_pool(name="psA", bufs=2, space="PSUM") as psA,
            tc.tile_pool(name="acc", bufs=2, space="PSUM") as accp,
            tc.tile_pool(name="aux", bufs=2, space="PSUM") as auxp,
            tc.tile_pool(name="hT", bufs=2, space="PSUM") as hTp,
        ):
            # ---- residents ----
            ident = res.tile([P, P], BF16)
            make_identity(nc, ident[:])
            wblk1 = res.tile([P, 8 * SLOTW], BF16)
            nc.sync.dma_start(wblk1[:], t_wblk1[:, :])
            wadb = res.tile([P, 8 * H1], BF16)
            nc.sync.dma_start(wadb[:], t_wadb[:, :])
            w1blk = res.tile([H1 * FIN, CW1], BF16)
            nc.sync.dma_start(w1blk[:], t_w1blk[:, :])
            waug2 = res.tile([CW1, C2 + 2], BF16)
            nc.sync.dma_start(waug2[:], t_waug2[:, :])
            wf = res.tile([C2, NCOUT], BF16)
            nc.sync.dma_start(wf[:], t_wf[:, :])
            a2rep = res.tile([P, C2], BF16)
            nc.sync.dma_start(a2rep[:], t_a2rep[:, :])
            b1T = res.tile([CW1, 1], F32)
            nc.sync.dma_start(b1T[:], t_b1T[:, :])
            b2r = res.tile([P, C2], F32)
            nc.sync.dma_start(b2r[:], t_b2r[:, :])
            bfr = res.tile([P, NCOUT], F32)
            nc.sync.dma_start(bfr[:], t_bfr[:, :])
            idx16 = res.tile([P, S * 8], I16)
            nc.sync.dma_start(idx16[:], t_idx16[:, :])
            m4 = res.tile([P, S * 4], BF16)
            nc.sync.dma_start(m4[:], t_m4[:, :])
            mask1 = res.tile([P, S], BF16)
            nc.sync.dma_start(mask1[:], t_mask1[:, :])
            xown = res.tile([P, cfg.TPC8 * P], BF16)
            nc.sync.dma_start(xown[:], t_xown[:, :])
            ad1 = res.tile([P, TPC * H1], F32)
            ad2 = res.tile([P, TPC], F32)

            # ---- alpha_dst1 for own nodes (stacked: 8 tiles per matmul) ----
            for I in range(cfg.TPC8):
                ps = psA.tile([P, 8 * SLOTW], F32, tag="ps_big")
                nc.tensor.matmul(ps[:, :8 * H1], lhsT=xown[:, I * P:(I + 1) * P],
                                 rhs=wadb[:], start=True, stop=True)
                ntt = min(8, TPC - I * 8)
                nc.vector.tensor_copy(ad1[:, I * 8 * H1:(I * 8 + ntt) * H1],
                                      ps[:, :ntt * H1])

            # ================= LAYER 1 + table2 build =================
            sp8 = 0    # padded slot offset (blocks)
            for gi, (t0, nt, Jg) in enumerate(groups1):
                SJ = nt * Jg
                SJ8 = SJ8s[gi]
                nblk = SJ8 // 8
                o = offs[t0]

                xe = pa.tile([P, 12 * P], BF16, tag="xe")
                nc.sync.dma_start(xe[:, :nblk * P],
                                  t_xstk[:, sp8 * 16:(sp8 + nblk * 8) * 16])
                sc = pb.tile([P, 96 * SLOTW], BF16, tag="sc")
                for b in range(nblk):
                    ps = psA.tile([P, 8 * SLOTW], F32, tag="ps_big")
                    nc.tensor.matmul(ps[:], lhsT=xe[:, b * P:(b + 1) * P],
                                     rhs=wblk1[:], start=True, stop=True)
                    if b % 2 == 0:
                        nc.scalar.activation(sc[:, b * 8 * SLOTW:(b + 1) * 8 * SLOTW],
                                             ps[:], AF.Copy)
                    else:
                        nc.vector.tensor_copy(sc[:, b * 8 * SLOTW:(b + 1) * 8 * SLOTW],
                                              ps[:])

                # scores: s = alpha_s + ad1 ; p = exp(lrelu(s)) * mask1
                scv = sc[:, :SJ * SLOTW].rearrange("p (s w) -> p s w", w=SLOTW)
                s1 = pb.tile([P, 96 * H1], F32, tag="s1")
                s4 = s1[:, :SJ * H1].rearrange("p (t j h) -> p t j h", j=Jg, h=H1)
                adv = ad1[:].rearrange("p (t h) -> p t h", h=H1)[:, t0:t0 + nt, :]
                nc.vector.tensor_tensor(
                    out=s4,
                    in0=scv[:, :, 0:H1].rearrange("p (t j) h -> p t j h", j=Jg),
                    in1=adv[:, :, None, :].to_broadcast([P, nt, Jg, H1]),
                    op=ALU.add)
                r1 = pb.tile([P, 96 * H1], F32, tag="r1")
                nc.scalar.activation(r1[:, :SJ * H1], s1[:, :SJ * H1], AF.Relu)
                nc.vector.tensor_scalar(out=r1[:, :SJ * H1], in0=r1[:, :SJ * H1],
                                        scalar1=4.0, scalar2=None, op0=ALU.mult)
                nc.vector.tensor_tensor(out=s1[:, :SJ * H1], in0=s1[:, :SJ * H1],
                                        in1=r1[:, :SJ * H1], op=ALU.add)
                pt = pb.tile([P, 96 * H1], F32, tag="pt")
                nc.scalar.activation(pt[:, :SJ * H1], s1[:, :SJ * H1], AF.Exp,
                                     scale=0.2)
                rhs2 = pb.tile([P, 96 * FV1], BF16, tag="rhs2")
                r2 = rhs2[:, :SJ * FV1].rearrange("p (s f) -> p s f", f=FV1)
                nc.vector.tensor_tensor(
                    out=r2[:, :, 0:H1],
                    in0=pt[:, :SJ * H1].rearrange("p (s h) -> p s h", h=H1),
                    in1=mask1[:, o:o + SJ][:, :, None].to_broadcast([P, SJ, H1]),
                    op=ALU.mult)
                # px: rhs2[:, s, 3+h*16+f] = x[s,f] * p[s,h]
                nc.vector.tensor_tensor(
                    out=r2[:, :, H1:].rearrange("p s (h f) -> p s h f", h=H1),
                    in0=scv[:, :, None, H1:].to_broadcast([P, SJ, H1, FIN]),
                    in1=r2[:, :, 0:H1].to_broadcast([P, SJ, H1, FIN]),
                    op=ALU.mult)

                # aggregate over j (PSUM accumulation)
                acc = accp.tile([P, NT1 * FV1], F32, tag="agg")
                for j in range(Jg):
                    nc.tensor.matmul(
                        acc[:, :nt * FV1], lhsT=ident[:],
                        rhs=rhs2[:, :SJ * FV1].rearrange(
                            "p (t j f) -> p t (j f)", j=Jg,
                            f=FV1)[:, :, j * FV1:(j + 1) * FV1],
                        start=(j == 0), stop=(j == Jg - 1))

                # z, reciprocal, aggx/z (bf16)
                av = acc[:, :nt * FV1].rearrange("p (t f) -> p t f", f=FV1)
                zr = fin.tile([P, NT1 * H1], F32, tag="zr")
                nc.vector.tensor_scalar(
                    out=zr[:, :nt * H1].rearrange("p (t h) -> p t h", h=H1),
                    in0=av[:, :, 0:H1], scalar1=1e-16, scalar2=None, op0=ALU.add)
                nc.vector.reciprocal(zr[:, :nt * H1], zr[:, :nt * H1])
                axz = fin.tile([P, NT1 * H1 * FIN], BF16, tag="axz")
                nc.vector.tensor_tensor(
                    out=axz[:, :nt * H1 * FIN].rearrange(
                        "p (t h f) -> p t h f", h=H1, f=FIN),
                    in0=av[:, :, H1:].rearrange("p t (h f) -> p t h f", h=H1),
                    in1=zr[:, :nt * H1].rearrange(
                        "p (t h) -> p t h", h=H1).to_broadcast([P, nt, H1, FIN]),
                    op=ALU.mult)

                # per tile: transpose -> blockdiag W1 -> h1T [96,128]
                hseg = fin.tile([CW1, NT1 * P], BF16, tag="hseg")
                for i in range(nt):
                    tp = auxp.tile([H1 * FIN, P], BF16, tag="tp")
                    nc.tensor.transpose(tp[:], axz[:, i * H1 * FIN:(i + 1) * H1 * FIN],
                                        ident[:])
                    axzT = pa.tile([H1 * FIN, P], BF16, tag="axzT")
                    nc.scalar.activation(axzT[:], tp[:], AF.Copy)
                    h1T = hTp.tile([CW1, P], F32, tag="h1T")
                    nc.tensor.matmul(h1T[:], lhsT=w1blk[:], rhs=axzT[:],
                                     start=True, stop=True)
                    nc.vector.tensor_tensor(
                        out=hseg[:, i * P:(i + 1) * P], in0=h1T[:],
                        in1=b1T[:, 0:1].to_broadcast([CW1, P]), op=ALU.add)
                # selu on [96, nt*128] -> h2T bf16
                h2T = pa.tile([CW1, NT1 * P], BF16, tag="h2T")
                rr = fin.tile([CW1, NT1 * P], BF16, tag="rrT")
                nc.scalar.activation(rr[:, :nt * P], hseg[:, :nt * P], AF.Relu)
                ww = fin.tile([CW1, NT1 * P], BF16, tag="wwT")
                nc.vector.tensor_tensor(out=ww[:, :nt * P], in0=hseg[:, :nt * P],
                                        in1=rr[:, :nt * P], op=ALU.subtract)
                nc.scalar.activation(ww[:, :nt * P], ww[:, :nt * P], AF.Exp)
                nc.vector.tensor_scalar(out=ww[:, :nt * P], in0=ww[:, :nt * P],
                                        scalar1=SELU_ALPHA_SCALE,
                                        scalar2=-SELU_ALPHA_SCALE,
                                        op0=ALU.mult, op1=ALU.add)
                nc.vector.tensor_scalar(out=rr[:, :nt * P], in0=rr[:, :nt * P],
                                        scalar1=SELU_SCALE, scalar2=None,
                                        op0=ALU.mult)
                nc.vector.tensor_tensor(out=h2T[:, :nt * P], in0=ww[:, :nt * P],
                                        in1=rr[:, :nt * P], op=ALU.add)

                # per tile: table2 row [as2|h2w|ad2] = h2T^T @ waug2
                st2 = fin.tile([P, NT1 * C2], BF16, tag="st2")
                for i in range(nt):
                    t = t0 + i
                    t2 = psA.tile([P, 8 * SLOTW], F32, tag="ps_big")
                    nc.tensor.matmul(t2[:, :C2 + 2], lhsT=h2T[:, i * P:(i + 1) * P],
                                     rhs=waug2[:], start=True, stop=True)
                    nc.vector.tensor_copy(ad2[:, t:t + 1], t2[:, C2 + 1:C2 + 2])
                    nc.scalar.activation(st2[:, i * C2:(i + 1) * C2],
                                         t2[:, 1:1 + C2], AF.Copy)
                dst_ap = t_cc_in[t0 * P:(t0 + nt) * P, :].rearrange(
                    "(i p) c -> p i c", p=P)
                nc.sync.dma_start(dst_ap, st2[:, :nt * C2].rearrange(
                    "p (i c) -> p i c", c=C2))
                sp8 += nblk * 8

                # chunked AllGather as soon as each half of the tiles is done
                if t0 + nt == cfg.T2:
                    nc.gpsimd.collective_compute(
                        "AllGather", ALU.bypass,
                        replica_groups=[list(range(cfg.ncores))],
                        ins=[t_cc_in[0:SH2, :].opt()],
                        outs=[t_cc_out[0:NQ // 2, :].opt()],
                    )
            nc.gpsimd.collective_compute(
                "AllGather", ALU.bypass,
                replica_groups=[list(range(cfg.ncores))],
                ins=[t_cc_in[SH2:SH, :].opt()],
                outs=[t_cc_out[NQ // 2:NQ, :].opt()],
            )

            # ================= LAYER 2 + head =================
            FV2 = 1 + C2  # 33
            for gi, (t0, nt, Jg) in enumerate(groups2):
                SJ = nt * Jg
                o = offs[t0]

                gath = pg.tile([P, 48 * 4 * C2], BF16, tag="gath")
                nidx = SJ * P
                base16 = o * 8
                cpos = 0
                while cpos < nidx:
                    ni = min(NI_MAX, nidx - cpos)
                    nc.gpsimd.dma_gather(
                        out_ap=gath[:, cpos:cpos + ni].rearrange(
                            "p (s e) -> p s e", e=4 * C2),
                        in_ap=t_cc_out[:, :],
                        idxs_ap=idx16[:, base16 + cpos // 16:base16 + (cpos + ni) // 16],
                        num_idxs=ni, num_idxs_reg=ni, elem_size=4 * C2,
                        queue_num=qctr[0] % 4)
                    qctr[0] += 1
                    cpos += ni

                # select sub-rows: gsel = gath * m4 (exactly one q live per slot)
                gsel = pb.tile([P, 48 * 4 * C2], BF16, tag="gsel")
                nc.vector.tensor_tensor(
                    out=gsel[:, :SJ * 4 * C2].rearrange("p (u c) -> p u c", c=C2),
                    in0=gath[:, :SJ * 4 * C2].rearrange("p (u c) -> p u c", c=C2),
                    in1=m4[:, o * 4:(o + SJ) * 4][:, :, None].to_broadcast(
                        [P, SJ * 4, C2]),
                    op=ALU.mult)
                g4 = gsel[:, :SJ * 4 * C2].rearrange("p (s u) -> p s u", u=4 * C2)
                vraw = pb.tile([P, 48 * C2], BF16, tag="vraw")
                vrv = vraw[:, :SJ * C2].rearrange("p (s c) -> p s c", c=C2)
                ta = pb.tile([P, 48 * C2], BF16, tag="ta")
                tav = ta[:, :SJ * C2].rearrange("p (s c) -> p s c", c=C2)
                nc.vector.tensor_tensor(out=tav, in0=g4[:, :, 0:C2],
                                        in1=g4[:, :, C2:2 * C2], op=ALU.add)
                nc.vector.tensor_tensor(out=vrv, in0=g4[:, :, 2 * C2:3 * C2],
                                        in1=g4[:, :, 3 * C2:4 * C2], op=ALU.add)
                nc.vector.tensor_tensor(out=vrv, in0=vrv, in1=tav, op=ALU.add)
                # alpha_src2 = vraw . a2
                gm2 = pb.tile([P, 48 * C2], BF16, tag="gm2")
                nc.vector.tensor_tensor(
                    out=gm2[:, :SJ * C2].rearrange("p (s c) -> p s c", c=C2),
                    in0=vrv, in1=a2rep[:][:, None, :].to_broadcast([P, SJ, C2]),
                    op=ALU.mult)
                as2 = pb.tile([P, 48], F32, tag="as2")
                nc.vector.tensor_reduce(
                    out=as2[:, :SJ],
                    in_=gm2[:, :SJ * C2].rearrange("p (s c) -> p s c", c=C2),
                    axis=AX.X, op=ALU.add)
                # p2 = exp(lrelu(as2 + ad2)) * mask1
                nc.vector.tensor_tensor(
                    out=as2[:, :SJ].rearrange("p (t j) -> p t j", j=Jg),
                    in0=as2[:, :SJ].rearrange("p (t j) -> p t j", j=Jg),
                    in1=ad2[:, t0:t0 + nt][:, :, None].to_broadcast([P, nt, Jg]),
                    op=ALU.add)
                r2t = pb.tile([P, 48], F32, tag="r2t")
                nc.scalar.activation(r2t[:, :SJ], as2[:, :SJ], AF.Relu)
                nc.vector.tensor_scalar(out=r2t[:, :SJ], in0=r2t[:, :SJ],
                                        scalar1=4.0, scalar2=None, op0=ALU.mult)
                nc.vector.tensor_tensor(out=as2[:, :SJ], in0=as2[:, :SJ],
                                        in1=r2t[:, :SJ], op=ALU.add)
                p2 = pb.tile([P, 48], F32, tag="p2")
                nc.scalar.activation(p2[:, :SJ], as2[:, :SJ], AF.Exp, scale=0.2)
                p2m = pb.tile([P, 48], BF16, tag="p2m")
                nc.vector.tensor_tensor(out=p2m[:, :SJ], in0=p2[:, :SJ],
                                        in1=mask1[:, o:o + SJ], op=ALU.mult)
                # rhs2b = [p2m | vraw * p2m]
                rhs2b = pb.tile([P, 48 * FV2], BF16, tag="rhs2b")
                rv = rhs2b[:, :SJ * FV2].rearrange("p (s f) -> p s f", f=FV2)
                nc.scalar.activation(rv[:, :, 0:1], p2m[:, :SJ][:, :, None], AF.Copy)
                nc.vector.tensor_tensor(
                    out=rv[:, :, 1:], in0=vrv,
                    in1=p2m[:, :SJ][:, :, None].to_broadcast([P, SJ, C2]),
                    op=ALU.mult)

                acc = accp.tile([P, NT1 * FV1], F32, tag="agg")
                for j in range(Jg):
                    nc.tensor.matmul(
                        acc[:, :nt * FV2], lhsT=ident[:],
                        rhs=rhs2b[:, :SJ * FV2].rearrange(
                            "p (t j f) -> p t (j f)", j=Jg,
                            f=FV2)[:, :, j * FV2:(j + 1) * FV2],
                        start=(j == 0), stop=(j == Jg - 1))

                av = acc[:, :nt * FV2].rearrange("p (t f) -> p t f", f=FV2)
                zr2 = fin.tile([P, NT2], F32, tag="zr2")
                nc.vector.tensor_scalar(
                    out=zr2[:, :nt][:, :, None],
                    in0=av[:, :, 0:1], scalar1=1e-16, scalar2=None, op0=ALU.add)
                nc.vector.reciprocal(zr2[:, :nt], zr2[:, :nt])
                h3f = fin.tile([P, NT2 * C2], F32, tag="h3f")
                h3v = h3f[:, :nt * C2].rearrange("p (t c) -> p t c", c=C2)
                nc.vector.tensor_tensor(
                    out=h3v, in0=av[:, :, 1:],
                    in1=zr2[:, :nt][:, :, None].to_broadcast([P, nt, C2]),
                    op=ALU.mult)
                nc.vector.tensor_tensor(
                    out=h3v, in0=h3v,
                    in1=b2r[:][:, None, :].to_broadcast([P, nt, C2]), op=ALU.add)
                # selu -> h3 bf16
                rr3 = fin.tile([P, NT2 * C2], F32, tag="rr3")
                nc.scalar.activation(rr3[:, :nt * C2], h3f[:, :nt * C2], AF.Relu)
                ww3 = fin.tile([P, NT2 * C2], F32, tag="ww3")
                nc.vector.tensor_tensor(out=ww3[:, :nt * C2], in0=h3f[:, :nt * C2],
                                        in1=rr3[:, :nt * C2], op=ALU.subtract)
                nc.scalar.activation(ww3[:, :nt * C2], ww3[:, :nt * C2], AF.Exp)
                nc.vector.tensor_scalar(out=ww3[:, :nt * C2], in0=ww3[:, :nt * C2],
                                        scalar1=SELU_ALPHA_SCALE,
                                        scalar2=-SELU_ALPHA_SCALE,
                                        op0=ALU.mult, op1=ALU.add)
                nc.vector.tensor_scalar(out=rr3[:, :nt * C2], in0=rr3[:, :nt * C2],
                                        scalar1=SELU_SCALE, scalar2=None,
                                        op0=ALU.mult)
                h3 = fin.tile([P, NT2 * C2], BF16, tag="h3")
                nc.vector.tensor_tensor(out=h3[:, :nt * C2], in0=ww3[:, :nt * C2],
                                        in1=rr3[:, :nt * C2], op=ALU.add)

                # final head per tile
                ost = fin.tile([P, NT2 * NCOUT], F32, tag="ost")
                for i in range(nt):
                    tp = auxp.tile([H1 * FIN, P], BF16, tag="tp")
                    nc.tensor.transpose(tp[:C2, :], h3[:, i * C2:(i + 1) * C2],
                                        ident[:])
                    h3T = pa.tile([C2, P], BF16, tag="h3T")
                    nc.scalar.activation(h3T[:], tp[:C2, :], AF.Copy)
                    pso = psA.tile([P, 8 * SLOTW], F32, tag="ps_big")
                    nc.tensor.matmul(pso[:, :NCOUT], lhsT=h3T[:], rhs=wf[:],
                                     start=True, stop=True)
                    nc.vector.tensor_tensor(out=ost[:, i * NCOUT:(i + 1) * NCOUT],
                                            in0=pso[:, :NCOUT], in1=bfr[:],
                                            op=ALU.add)
                dst_ap = t_out[t0 * P:(t0 + nt) * P, :].rearrange(
                    "(i p) c -> p i c", p=P)
                nc.sync.dma_start(dst_ap, ost[:, :nt * NCOUT].rearrange(
                    "p (i c) -> p i c", c=NCOUT))

    nc.compile()
    return nc


def _make_inputs(cfg, percore, wpack, inputs):
    b1 = np.asarray(inputs["b1"], np.float32)
    b2 = np.asarray(inputs["b2"], np.float32)
    bf = np.asarray(inputs["bf"], np.float32)
    wf = np.asarray(inputs["Wf"], np.float32).astype(NPBF16)
    b1T = np.ascontiguousarray(b1[:, None])
    b2r = np.broadcast_to(b2, (P, cfg.C2)).copy()
    bfr = np.broadcast_to(bf, (P, cfg.NCOUT)).copy()
    shared = {
        "waug_blk1": wpack["waug_blk1"].astype(NPBF16),
        "wad_blk": wpack["wad_blk"].astype(NPBF16),
        "w1blk": wpack["w1blk"].astype(NPBF16),
        "waug2": wpack["waug2"].astype(NPBF16),
        "a2rep": wpack["a2rep"].astype(NPBF16),
        "wf": wf, "b1T": b1T, "b2r": b2r, "bfr": bfr,
    }
    in_maps = []
    for c in range(cfg.ncores):
        pc = percore[c]
        m = dict(shared)
        m.update({"xstk": pc["xstk"], "xown_stk": pc["xown_stk"],
                  "idx16": pc["idx16"], "m4": pc["m4"], "mask1": pc["mask1"]})
        in_maps.append(m)
    return in_maps


def _assemble(cfg, percore, results):
    out = np.zeros((cfg.NPAD, cfg.NCOUT), np.float32)
    for c in range(cfg.ncores):
        out[percore[c]["own"]] = results[c]["out"]
    return out[:cfg.N]


def kernel(**inputs) -> np.ndarray:
    cfg = Cfg(N=100000, E=800000, ncores=8)
    percore, wpack, meta = preprocess(
        cfg,
        np.asarray(inputs["x"], np.float32),
        np.asarray(inputs["edge_index"]),
        np.asarray(inputs["W1"], np.float32),
        np.asarray(inputs["a_src1"], np.float32),
        np.asarray(inputs["a_dst1"], np.float32),
        np.asarray(inputs["W2"], np.float32),
        np.asarray(inputs["a_src2"], np.float32),
        np.asarray(inputs["a_dst2"], np.float32),
    )
    nc = build_nc(cfg, meta)
    in_maps = _make_inputs(cfg, percore, wpack, inputs)
    res = run_bass_kernel_spmd(nc, in_maps, core_ids=list(range(cfg.ncores)))
    return _assemble(cfg, percore, res.results)


if __name__ == "__main__":
    import reference as R
    inputs = R.setup_inputs()
    out = kernel(**{k: np.asarray(v) for k, v in inputs.items()})
    print("out", out.shape, out.dtype)


# revision 25
# speedup vs baseline: 1.4179x; 1.0040x over previous
"""Trainium2 Bass kernel for nn_GAT_1675037246077 (2-layer GAT + linear head).

Strategy (8 NeuronCores, SPMD single NEFF), v2:
 - Destination-sharded: core c owns dst nodes [c*12544, (c+1)*12544); nodes padded
   to NPAD = 100352. Own nodes degree-sorted into 98 tiles of 128; edge grid per
   tile [128 dst x J_t] with J_t = cross-core max degree (equal-J groups, nt<=8).
 - Layer 1 (no gathers): host pre-expands x into a *stacked* layout packing 8
   edge-slots into the 128 partitions; one matmul vs blockdiag([Wa|I16]x8)
   yields per-slot [alpha_src(3) | x(16)] for 8 slots at once. Weighted x
   (p*x, 48 cols) is aggregated over slots via identity-matmul PSUM chains,
   divided by z, then W1 applied once per tile (blockdiag over heads) producing
   h1^T [96,128] directly; bias+selu; table2 row h2w = selu(...)@W2 via one more
   matmul per tile. Pad slots are zeroed by host masks (no correction terms).
 - Table2: 32 bf16 cols (h2w only) = 64B rows; alpha_src2 recovered on device as
   h2w . a_src2; alpha_dst2 kept core-local. AllGather in 2 row-chunks
   (renumbered table rows) to overlap with layer-1 tail.
 - Layer 2 gather: dma_gather (InstDMAGatherAnt) of 256B quads (4 rows) from the
   quad-view table [NPAD/4, 128], int16 quad indices, round-robin across 4 SWDGE
   queues (4x descriptor-generation parallelism). Sub-row selection and softmax
   weights fold into one mask-multiply; per-(slot,q) weighted rows reduce over q
   on DVE, aggregate over slots via identity matmuls, divide by z, selu, final
   head matmul per tile.
"""
import numpy as np
import ml_dtypes

from concourse import mybir, tile, bacc
import concourse.bass as bass
from concourse.bass_utils import run_bass_kernel_spmd
from concourse.masks import make_identity

P = 128
AF = mybir.ActivationFunctionType
ALU = mybir.AluOpType
AX = mybir.AxisListType
BF16 = mybir.dt.bfloat16
F32 = mybir.dt.float32
I16 = mybir.dt.int16
NPBF16 = ml_dtypes.bfloat16

SELU_SCALE = 1.0507009873554805
SELU_ALPHA_SCALE = 1.7580993408473766  # scale * alpha
NT1, CAP1 = 10, 90    # layer-1 groups (PSUM: 10*51*4 = 2040B <= bank)
NT2, CAP2 = 8, 48     # layer-2 groups (gather tiles sized for SJ<=48)
NI_MAX = 1024         # max idxs per dma_gather instruction (HW limit)


class Cfg:
    def __init__(self, N, E, ncores, fin=16, h1=3, c1=32, c2=32, ncout=16):
        self.N, self.E, self.ncores = N, E, ncores
        self.FIN, self.H1, self.C1, self.C2, self.NCOUT = fin, h1, c1, c2, ncout
        self.CW1 = h1 * c1              # 96
        self.SH = ((N + ncores - 1) // ncores + P - 1) // P * P   # 12544
        self.TPC = self.SH // P                                    # 98
        self.NPAD = self.SH * ncores                               # 100352
        self.SH2 = self.SH // 2                                    # 6272
        self.T2 = self.TPC // 2                                    # 49 (chunk bnd)
        self.FV1 = h1 + h1 * fin        # 51: [p(3) | px(48)]
        self.SLOT1 = h1 + fin + h1 * fin  # not used directly
        self.TPC8 = (self.TPC + 7) // 8  # 13


def _group_plan(J, t_break, nt_max, sj_cap):
    """Runs of equal J, nt <= nt_max, nt*J <= sj_cap, forced break at t_break."""
    groups = []
    t = 0
    n = len(J)
    while t < n:
        j = J[t]
        assert j <= sj_cap, f"tile degree {j} exceeds cap {sj_cap}"
        nt = 1
        while (t + nt < n and J[t + nt] == j and nt < nt_max
               and (nt + 1) * j <= sj_cap and (t + nt) != t_break):
            nt += 1
        groups.append((t, nt, int(j)))
        t += nt
    return groups


def preprocess(cfg, x, edge_index, W1, a_src1, a_dst1, W2, a_src2, a_dst2):
    """Host-side sharding/packing. Returns (percore list, wpack dict, meta)."""
    N, E, NC = cfg.N, cfg.E, cfg.ncores
    SH, TPC, NPAD, SH2 = cfg.SH, cfg.TPC, cfg.NPAD, cfg.SH2
    FIN, H1, C1, C2, CW1 = cfg.FIN, cfg.H1, cfg.C1, cfg.C2, cfg.CW1

    loops = np.arange(N, dtype=np.int64)
    src = np.concatenate([edge_index[0].astype(np.int64), loops])
    dst = np.concatenate([edge_index[1].astype(np.int64), loops])

    deg = np.bincount(dst, minlength=NPAD)

    perms = []
    Jt_all = np.zeros((NC, TPC), np.int64)
    for c in range(NC):
        d = deg[c * SH:(c + 1) * SH]
        order = np.argsort(-d, kind="stable")
        perm = c * SH + order
        perms.append(perm)
        Jt_all[c] = d[order].reshape(TPC, P).max(1)
    J = np.maximum(Jt_all.max(0), 1)
    S = int(J.sum())

    # table-row numbering for 2-chunk AllGather:
    # slot < SH2: row = c*SH2 + slot ; else row = NPAD/2 + c*SH2 + (slot-SH2)
    pos2 = np.empty(NPAD, np.int64)
    for c in range(NC):
        slot = np.arange(SH)
        row = np.where(slot < SH2, c * SH2 + slot,
                       NPAD // 2 + c * SH2 + (slot - SH2))
        pos2[perms[c]] = row

    # CSR of edges by dst
    e_order = np.argsort(dst, kind="stable")
    src_sorted = src[e_order]
    starts = np.zeros(NPAD + 1, np.int64)
    np.cumsum(deg, out=starts[1:])

    offs = np.zeros(TPC + 1, np.int64)
    np.cumsum(J, out=offs[1:])

    groups1 = _group_plan(J, cfg.T2, NT1, CAP1)
    groups2 = _group_plan(J, cfg.T2, NT2, CAP2)
    # padded-slot stream for L1 stacking
    SJ8s = []
    for (t0, nt, Jg) in groups1:
        SJ = nt * Jg
        SJ8s.append((SJ + 7) // 8 * 8)
    S8 = int(sum(SJ8s))
    NBLK = S8 // 8

    xpad = np.zeros((NPAD, FIN), np.float32)
    xpad[:N] = x
    xpadT_bf = np.ascontiguousarray(xpad.T).astype(NPBF16)

    percore = []
    for c in range(NC):
        perm = perms[c]
        idx1 = np.full((P, S), NPAD - 1, np.int64)   # src node per slot (pad->zero row)
        real = np.zeros((P, S), bool)
        for t in range(TPC):
            jt = int(J[t])
            o = int(offs[t])
            for p in range(P):
                node = perm[t * P + p]
                dg = int(deg[node])
                s0 = int(starts[node])
                idx1[p, o:o + dg] = src_sorted[s0:s0 + dg]
                real[p, o:o + dg] = True

        # --- L1 stacked x expansion ---
        idxpad = np.full((P, S8), NPAD - 1, np.int64)
        sp = 0
        for gi, (t0, nt, Jg) in enumerate(groups1):
            SJ = nt * Jg
            o = int(offs[t0])
            idxpad[:, sp:sp + SJ] = idx1[:, o:o + SJ]
            sp += SJ8s[gi]
        G = xpadT_bf[:, idxpad.T.ravel()]                     # [16, S8*128]
        G = G.reshape(FIN, NBLK, 8, P).transpose(2, 0, 1, 3)  # [8,16,NBLK,128]
        xstk = np.ascontiguousarray(G.reshape(P, NBLK * P))

        # --- own-node stacked x (for alpha_dst) ---
        perm_pad = np.concatenate(
            [perm, np.full(cfg.TPC8 * 8 * P - SH, NPAD - 1, np.int64)])
        G2 = xpadT_bf[:, perm_pad].reshape(FIN, cfg.TPC8, 8, P)
        xown_stk = np.ascontiguousarray(
            G2.transpose(2, 0, 1, 3).reshape(P, cfg.TPC8 * P))

        # --- L2 quad indices + masks ---
        idx2 = pos2[idx1]                      # [P, S]
        idx2[~real] = 0
        qidx = (idx2 // 4).astype(np.int16)
        sub = (idx2 % 4).astype(np.int64)
        m4 = np.zeros((P, S, 4), NPBF16)
        m4[np.arange(P)[:, None], np.arange(S)[None, :], sub] = real.astype(NPBF16)
        m4 = np.ascontiguousarray(m4.reshape(P, S * 4))
        mask1 = np.ascontiguousarray(real.astype(NPBF16))

        # idx16 stream per L2 group: i = s_local*128 + d -> [i%16, i//16]
        chunks = []
        for (t0, nt, Jg) in groups2:
            SJ = nt * Jg
            o = int(offs[t0])
            stream = qidx[:, o:o + SJ].T.ravel()   # [SJ*128], i = s*128+d
            chunks.append(stream.reshape(-1, 16).T)  # [16, SJ*8]
        idx16 = np.concatenate(chunks, axis=1)       # [16, S*8]
        idx16 = np.ascontiguousarray(np.tile(idx16, (8, 1)))  # [128, S*8]

        percore.append(dict(own=perm.astype(np.int64), xstk=xstk,
                            xown_stk=xown_stk, idx16=idx16, m4=m4, mask1=mask1))

    # ---- weights ----
    wa1 = np.zeros((FIN, H1), np.float32)
    wad1 = np.zeros((FIN, H1), np.float32)
    for h in range(H1):
        wa1[:, h] = W1[:, h * C1:(h + 1) * C1] @ a_src1[h]
        wad1[:, h] = W1[:, h * C1:(h + 1) * C1] @ a_dst1[h]
    waug_blk1 = np.zeros((P, 8 * (H1 + FIN)), np.float32)   # [128, 152]
    wad_blk = np.zeros((P, 8 * H1), np.float32)             # [128, 24]
    for s in range(8):
        r = s * FIN
        cbase = s * (H1 + FIN)
        waug_blk1[r:r + FIN, cbase:cbase + H1] = wa1
        waug_blk1[r:r + FIN, cbase + H1:cbase + H1 + FIN] = np.eye(FIN)
        wad_blk[r:r + FIN, s * H1:(s + 1) * H1] = wad1
    w1blk = np.zeros((H1 * FIN, CW1), np.float32)           # [48, 96]
    for h in range(H1):
        w1blk[h * FIN:(h + 1) * FIN, h * C1:(h + 1) * C1] = \
            W1[:, h * C1:(h + 1) * C1]
    waug2 = np.zeros((CW1, C2 + 2), np.float32)             # [96, 34]
    waug2[:, 0] = W2 @ a_src2[0]
    waug2[:, 1:1 + C2] = W2
    waug2[:, 1 + C2] = W2 @ a_dst2[0]
    a2rep = np.broadcast_to(a_src2[0].astype(np.float32), (P, C2)).copy()

    wpack = dict(waug_blk1=waug_blk1, wad_blk=wad_blk, w1blk=w1blk,
                 waug2=waug2, a2rep=a2rep)
    meta = dict(J=[int(j) for j in J], offs=[int(o) for o in offs],
                S=S, S8=S8, NBLK=NBLK, groups=groups1, groups1=groups1,
                groups2=groups2, SJ8s=SJ8s)
    return percore, wpack, meta


def build_nc(cfg, meta):
    J, offs, SJ8s = meta["J"], meta["offs"], meta["SJ8s"]
    groups1, groups2 = meta["groups1"], meta["groups2"]
    S, S8, NBLK = meta["S"], meta["S8"], meta["NBLK"]
    TPC, NPAD, SH, SH2 = cfg.TPC, cfg.NPAD, cfg.SH, cfg.SH2
    FIN, H1, CW1, C2, NCOUT = cfg.FIN, cfg.H1, cfg.CW1, cfg.C2, cfg.NCOUT
    FV1 = cfg.FV1                     # 51
    SLOTW = H1 + FIN                  # 19
    NQ = NPAD // 4

    nc = bacc.Bacc("TRN2", target_bir_lowering=False,
                   num_devices=cfg.ncores, num_swdge_queues=4)

    # ---- I/O ----
    t_xstk = nc.dram_tensor("xstk", [P, NBLK * P], BF16, kind="ExternalInput")
    t_xown = nc.dram_tensor("xown_stk", [P, cfg.TPC8 * P], BF16, kind="ExternalInput")
    t_idx16 = nc.dram_tensor("idx16", [P, S * 8], I16, kind="ExternalInput")
    t_m4 = nc.dram_tensor("m4", [P, S * 4], BF16, kind="ExternalInput")
    t_mask1 = nc.dram_tensor("mask1", [P, S], BF16, kind="ExternalInput")
    t_wblk1 = nc.dram_tensor("waug_blk1", [P, 8 * SLOTW], BF16, kind="ExternalInput")
    t_wadb = nc.dram_tensor("wad_blk", [P, 8 * H1], BF16, kind="ExternalInput")
    t_w1blk = nc.dram_tensor("w1blk", [H1 * FIN, CW1], BF16, kind="ExternalInput")
    t_waug2 = nc.dram_tensor("waug2", [CW1, C2 + 2], BF16, kind="ExternalInput")
    t_wf = nc.dram_tensor("wf", [C2, NCOUT], BF16, kind="ExternalInput")
    t_a2rep = nc.dram_tensor("a2rep", [P, C2], BF16, kind="ExternalInput")
    t_b1T = nc.dram_tensor("b1T", [CW1, 1], F32, kind="ExternalInput")
    t_b2r = nc.dram_tensor("b2r", [P, C2], F32, kind="ExternalInput")
    t_bfr = nc.dram_tensor("bfr", [P, NCOUT], F32, kind="ExternalInput")
    t_out = nc.dram_tensor("out", [SH, NCOUT], F32, kind="ExternalOutput")

    t_cc_in = nc.dram_tensor("cc_in", [SH, C2], BF16)
    cc_space = "Shared" if cfg.ncores > 4 else "Local"
    t_cc_out = nc.dram_tensor("cc_out", [NQ, 4 * C2], BF16, addr_space=cc_space)

    qctr = [0]

    with tile.TileContext(nc) as tc:
        with (
            tc.tile_pool(name="res", bufs=1) as res,
            tc.tile_pool(name="pa", bufs=3) as pa,
            tc.tile_pool(name="pb", bufs=2) as pb,
            tc.tile_pool(name="pg", bufs=3) as pg,
            tc.tile_pool(name="fin", bufs=2) as fin,
            tc.tile_pool(name="psA", bufs=2, space="PSUM") as psA,
            tc.tile_pool(name="acc", bufs=2, space="PSUM") as accp,
            tc.tile_pool(name="aux", bufs=2, space="PSUM") as auxp,
            tc.tile_pool(name="hT", bufs=2, space="PSUM") as hTp,
        ):
            # ---- residents ----
            ident = res.tile([P, P], BF16)
            make_identity(nc, ident[:])
            wblk1 = res.tile([P, 8 * SLOTW], BF16)
            nc.sync.dma_start(wblk1[:], t_wblk1[:, :])
            wadb = res.tile([P, 8 * H1], BF16)
            nc.sync.dma_start(wadb[:], t_wadb[:, :])
            w1blk = res.tile([H1 * FIN, CW1], BF16)
            nc.sync.dma_start(w1blk[:], t_w1blk[:, :])
            waug2 = res.tile([CW1, C2 + 2], BF16)
            nc.sync.dma_start(waug2[:], t_waug2[:, :])
            wf = res.tile([C2, NCOUT], BF16)
            nc.sync.dma_start(wf[:], t_wf[:, :])
            a2rep = res.tile([P, C2], BF16)
            nc.sync.dma_start(a2rep[:], t_a2rep[:, :])
            b1T = res.tile([CW1, 1], F32)
            nc.sync.dma_start(b1T[:], t_b1T[:, :])
            b2r = res.tile([P, C2], F32)
            nc.sync.dma_start(b2r[:], t_b2r[:, :])
            bfr = res.tile([P, NCOUT], F32)
            nc.sync.dma_start(bfr[:], t_bfr[:, :])
            idx16 = res.tile([P, S * 8], I16)
            nc.sync.dma_start(idx16[:], t_idx16[:, :])
            m4 = res.tile([P, S * 4], BF16)
            nc.sync.dma_start(m4[:], t_m4[:, :])
            mask1 = res.tile([P, S], BF16)
            nc.sync.dma_start(mask1[:], t_mask1[:, :])
            xown = res.tile([P, cfg.TPC8 * P], BF16)
            nc.sync.dma_start(xown[:], t_xown[:, :])
            ad1 = res.tile([P, TPC * H1], F32)
            adx = res.tile([P, TPC * 33], BF16)  # [h2w(32)|ad2(1)] per tile
            ad2v = adx[:].rearrange("p (t c) -> p t c", c=33)

            # ---- alpha_dst1 for own nodes (stacked: 8 tiles per matmul) ----
            for I in range(cfg.TPC8):
                ps = psA.tile([P, 8 * SLOTW], F32, tag="ps_big")
                nc.tensor.matmul(ps[:, :8 * H1], lhsT=xown[:, I * P:(I + 1) * P],
                                 rhs=wadb[:], start=True, stop=True)
                ntt = min(8, TPC - I * 8)
                nc.vector.tensor_copy(ad1[:, I * 8 * H1:(I * 8 + ntt) * H1],
                                      ps[:, :ntt * H1])

            # ================= LAYER 1 + table2 build =================
            sp8 = 0    # padded slot offset (blocks)
            for gi, (t0, nt, Jg) in enumerate(groups1):
                SJ = nt * Jg
                SJ8 = SJ8s[gi]
                nblk = SJ8 // 8
                o = offs[t0]

                xe = pa.tile([P, 12 * P], BF16, tag="xe")
                nc.sync.dma_start(xe[:, :nblk * P],
                                  t_xstk[:, sp8 * 16:(sp8 + nblk * 8) * 16])
                sc = pb.tile([P, 96 * SLOTW], BF16, tag="sc")
                for b in range(nblk):
                    ps = psA.tile([P, 8 * SLOTW], F32, tag="ps_big")
                    nc.tensor.matmul(ps[:], lhsT=xe[:, b * P:(b + 1) * P],
                                     rhs=wblk1[:], start=True, stop=True)
                    if b % 2 == 0:
                        nc.scalar.activation(sc[:, b * 8 * SLOTW:(b + 1) * 8 * SLOTW],
                                             ps[:], AF.Copy)
                    else:
                        nc.vector.tensor_copy(sc[:, b * 8 * SLOTW:(b + 1) * 8 * SLOTW],
                                              ps[:])

                # scores: s = alpha_s + ad1 ; p = exp(lrelu(s)) * mask1
                scv = sc[:, :SJ * SLOTW].rearrange("p (s w) -> p s w", w=SLOTW)
                s1 = pb.tile([P, 96 * H1], F32, tag="s1")
                s4 = s1[:, :SJ * H1].rearrange("p (t j h) -> p t j h", j=Jg, h=H1)
                adv = ad1[:].rearrange("p (t h) -> p t h", h=H1)[:, t0:t0 + nt, :]
                nc.vector.tensor_tensor(
                    out=s4,
                    in0=scv[:, :, 0:H1].rearrange("p (t j) h -> p t j h", j=Jg),
                    in1=adv[:, :, None, :].to_broadcast([P, nt, Jg, H1]),
                    op=ALU.add)
                r1 = pb.tile([P, 96 * H1], F32, tag="r1")
                nc.vector.tensor_scalar(out=r1[:, :SJ * H1], in0=s1[:, :SJ * H1],
                                        scalar1=5.0, scalar2=None, op0=ALU.mult)
                nc.vector.tensor_tensor(out=s1[:, :SJ * H1], in0=s1[:, :SJ * H1],
                                        in1=r1[:, :SJ * H1], op=ALU.max)
                pt = pb.tile([P, 96 * H1], F32, tag="pt")
                nc.scalar.activation(pt[:, :SJ * H1], s1[:, :SJ * H1], AF.Exp,
                                     scale=0.2)
                rhs2 = pb.tile([P, 96 * FV1], BF16, tag="rhs2")
                r2 = rhs2[:, :SJ * FV1].rearrange("p (s f) -> p s f", f=FV1)
                nc.vector.tensor_tensor(
                    out=r2[:, :, 0:H1],
                    in0=pt[:, :SJ * H1].rearrange("p (s h) -> p s h", h=H1),
                    in1=mask1[:, o:o + SJ][:, :, None].to_broadcast([P, SJ, H1]),
                    op=ALU.mult)
                # px: rhs2[:, s, 3+h*16+f] = x[s,f] * p[s,h]
                nc.vector.tensor_tensor(
                    out=r2[:, :, H1:].rearrange("p s (h f) -> p s h f", h=H1),
                    in0=scv[:, :, None, H1:].to_broadcast([P, SJ, H1, FIN]),
                    in1=r2[:, :, 0:H1].to_broadcast([P, SJ, H1, FIN]),
                    op=ALU.mult)

                # aggregate over j (PSUM accumulation)
                acc = accp.tile([P, NT1 * FV1], F32, tag="agg")
                for j in range(Jg):
                    nc.tensor.matmul(
                        acc[:, :nt * FV1], lhsT=ident[:],
                        rhs=rhs2[:, :SJ * FV1].rearrange(
                            "p (t j f) -> p t (j f)", j=Jg,
                            f=FV1)[:, :, j * FV1:(j + 1) * FV1],
                        start=(j == 0), stop=(j == Jg - 1))

                # z, reciprocal, aggx/z (bf16)
                av = acc[:, :nt * FV1].rearrange("p (t f) -> p t f", f=FV1)
                zr = fin.tile([P, NT1 * H1], F32, tag="zr")
                nc.vector.tensor_scalar(
                    out=zr[:, :nt * H1].rearrange("p (t h) -> p t h", h=H1),
                    in0=av[:, :, 0:H1], scalar1=1e-16, scalar2=None, op0=ALU.add)
                nc.vector.reciprocal(zr[:, :nt * H1], zr[:, :nt * H1])
                axz = fin.tile([P, NT1 * H1 * FIN], BF16, tag="axz")
                nc.vector.tensor_tensor(
                    out=axz[:, :nt * H1 * FIN].rearrange(
                        "p (t h f) -> p t h f", h=H1, f=FIN),
                    in0=av[:, :, H1:].rearrange("p t (h f) -> p t h f", h=H1),
                    in1=zr[:, :nt * H1].rearrange(
                        "p (t h) -> p t h", h=H1).to_broadcast([P, nt, H1, FIN]),
                    op=ALU.mult)

                # per tile: transpose -> blockdiag W1 -> h1T [96,128]
                hseg = fin.tile([CW1, NT1 * P], BF16, tag="hseg")
                for i in range(nt):
                    tp = auxp.tile([H1 * FIN, P], BF16, tag="tp")
                    nc.tensor.transpose(tp[:], axz[:, i * H1 * FIN:(i + 1) * H1 * FIN],
                                        ident[:])
                    axzT = pa.tile([H1 * FIN, P], BF16, tag="axzT")
                    nc.scalar.activation(axzT[:], tp[:], AF.Copy)
                    h1T = hTp.tile([CW1, P], F32, tag="h1T")
                    nc.tensor.matmul(h1T[:], lhsT=w1blk[:], rhs=axzT[:],
                                     start=True, stop=True)
                    nc.scalar.activation(hseg[:, i * P:(i + 1) * P], h1T[:],
                                         AF.Identity, bias=b1T[:, 0:1])
                # selu on [96, nt*128] -> h2T bf16
                h2T = pa.tile([CW1, NT1 * P], BF16, tag="h2T")
                rr = fin.tile([CW1, NT1 * P], BF16, tag="rrT")
                nc.scalar.activation(rr[:, :nt * P], hseg[:, :nt * P], AF.Relu)
                ww = fin.tile([CW1, NT1 * P], BF16, tag="wwT")
                nc.vector.tensor_tensor(out=ww[:, :nt * P], in0=hseg[:, :nt * P],
                                        in1=rr[:, :nt * P], op=ALU.subtract)
                nc.scalar.activation(ww[:, :nt * P], ww[:, :nt * P], AF.Exp)
                nc.vector.tensor_scalar(out=ww[:, :nt * P], in0=ww[:, :nt * P],
                                        scalar1=SELU_ALPHA_SCALE,
                                        scalar2=-SELU_ALPHA_SCALE,
                                        op0=ALU.mult, op1=ALU.add)
                nc.vector.tensor_scalar(out=rr[:, :nt * P], in0=rr[:, :nt * P],
                                        scalar1=SELU_SCALE, scalar2=None,
                                        op0=ALU.mult)
                nc.vector.tensor_tensor(out=h2T[:, :nt * P], in0=ww[:, :nt * P],
                                        in1=rr[:, :nt * P], op=ALU.add)

                # per tile: table2 row [as2|h2w|ad2] = h2T^T @ waug2
                for i in range(nt):
                    t = t0 + i
                    t2 = psA.tile([P, 8 * SLOTW], F32, tag="ps_big")
                    nc.tensor.matmul(t2[:, :C2 + 2], lhsT=h2T[:, i * P:(i + 1) * P],
                                     rhs=waug2[:], start=True, stop=True)
                    nc.scalar.activation(adx[:, t * 33:t * 33 + 33],
                                         t2[:, 1:C2 + 2], AF.Copy)
                dst_ap = t_cc_in[t0 * P:(t0 + nt) * P, :].rearrange(
                    "(i p) c -> p i c", p=P)
                nc.sync.dma_start(dst_ap, adx[:, t0 * 33:(t0 + nt) * 33].rearrange(
                    "p (i c) -> p i c", c=33)[:, :, 0:C2])
                sp8 += nblk * 8

                # chunked AllGather as soon as each half of the tiles is done
                if t0 + nt == cfg.T2:
                    nc.gpsimd.collective_compute(
                        "AllGather", ALU.bypass,
                        replica_groups=[list(range(cfg.ncores))],
                        ins=[t_cc_in[0:SH2, :].opt()],
                        outs=[t_cc_out[0:NQ // 2, :].opt()],
                    )
            nc.gpsimd.collective_compute(
                "AllGather", ALU.bypass,
                replica_groups=[list(range(cfg.ncores))],
                ins=[t_cc_in[SH2:SH, :].opt()],
                outs=[t_cc_out[NQ // 2:NQ, :].opt()],
            )

            # ================= LAYER 2 + head =================
            FV2 = 1 + C2  # 33
            for gi, (t0, nt, Jg) in enumerate(groups2):
                SJ = nt * Jg
                o = offs[t0]

                gath = pg.tile([P, 48 * 4 * C2], BF16, tag="gath")
                nidx = SJ * P
                base16 = o * 8
                cpos = 0
                while cpos < nidx:
                    ni = min(NI_MAX, nidx - cpos)
                    nc.gpsimd.dma_gather(
                        out_ap=gath[:, cpos:cpos + ni].rearrange(
                            "p (s e) -> p s e", e=4 * C2),
                        in_ap=t_cc_out[:, :],
                        idxs_ap=idx16[:, base16 + cpos // 16:base16 + (cpos + ni) // 16],
                        num_idxs=ni, num_idxs_reg=ni, elem_size=4 * C2,
                        queue_num=qctr[0] % 4)
                    qctr[0] += 1
                    cpos += ni

                # select sub-rows in place: gath *= m4 (one q live per slot)
                nc.vector.tensor_tensor(
                    out=gath[:, :SJ * 4 * C2].rearrange("p (u c) -> p u c", c=C2),
                    in0=gath[:, :SJ * 4 * C2].rearrange("p (u c) -> p u c", c=C2),
                    in1=m4[:, o * 4:(o + SJ) * 4][:, :, None].to_broadcast(
                        [P, SJ * 4, C2]),
                    op=ALU.mult)
                g4 = gath[:, :SJ * 4 * C2].rearrange("p (s u) -> p s u", u=4 * C2)
                vraw = pb.tile([P, 48 * C2], BF16, tag="vraw")
                vrv = vraw[:, :SJ * C2].rearrange("p (s c) -> p s c", c=C2)
                ta = pb.tile([P, 48 * C2], BF16, tag="ta")
                tav = ta[:, :SJ * C2].rearrange("p (s c) -> p s c", c=C2)
                nc.vector.tensor_tensor(out=tav, in0=g4[:, :, 0:C2],
                                        in1=g4[:, :, C2:2 * C2], op=ALU.add)
                nc.vector.tensor_tensor(out=vrv, in0=g4[:, :, 2 * C2:3 * C2],
                                        in1=g4[:, :, 3 * C2:4 * C2], op=ALU.add)
                nc.vector.tensor_tensor(out=vrv, in0=vrv, in1=tav, op=ALU.add)
                # alpha_src2 = vraw . a2
                gm2 = pb.tile([P, 48 * C2], BF16, tag="gm2")
                nc.vector.tensor_tensor(
                    out=gm2[:, :SJ * C2].rearrange("p (s c) -> p s c", c=C2),
                    in0=vrv, in1=a2rep[:][:, None, :].to_broadcast([P, SJ, C2]),
                    op=ALU.mult)
                as2 = pb.tile([P, 48], F32, tag="as2")
                nc.vector.tensor_reduce(
                    out=as2[:, :SJ],
                    in_=gm2[:, :SJ * C2].rearrange("p (s c) -> p s c", c=C2),
                    axis=AX.X, op=ALU.add)
                # p2 = exp(lrelu(as2 + ad2)) * mask1
                nc.vector.tensor_tensor(
                    out=as2[:, :SJ].rearrange("p (t j) -> p t j", j=Jg),
                    in0=as2[:, :SJ].rearrange("p (t j) -> p t j", j=Jg),
                    in1=ad2v[:, t0:t0 + nt, 32:33].to_broadcast([P, nt, Jg]),
                    op=ALU.add)
                r2t = pb.tile([P, 48], F32, tag="r2t")
                nc.vector.tensor_scalar(out=r2t[:, :SJ], in0=as2[:, :SJ],
                                        scalar1=5.0, scalar2=None, op0=ALU.mult)
                nc.vector.tensor_tensor(out=as2[:, :SJ], in0=as2[:, :SJ],
                                        in1=r2t[:, :SJ], op=ALU.max)
                p2 = pb.tile([P, 48], F32, tag="p2")
                nc.scalar.activation(p2[:, :SJ], as2[:, :SJ], AF.Exp, scale=0.2)
                p2m = pb.tile([P, 48], BF16, tag="p2m")
                nc.vector.tensor_tensor(out=p2m[:, :SJ], in0=p2[:, :SJ],
                                        in1=mask1[:, o:o + SJ], op=ALU.mult)
                # rhs2b = [p2m | vraw * p2m]
                rhs2b = pb.tile([P, 48 * FV2], BF16, tag="rhs2b")
                rv = rhs2b[:, :SJ * FV2].rearrange("p (s f) -> p s f", f=FV2)
                nc.scalar.activation(rv[:, :, 0:1], p2m[:, :SJ][:, :, None], AF.Copy)
                nc.vector.tensor_tensor(
                    out=rv[:, :, 1:], in0=vrv,
                    in1=p2m[:, :SJ][:, :, None].to_broadcast([P, SJ, C2]),
                    op=ALU.mult)

                acc = accp.tile([P, NT1 * FV1], F32, tag="agg")
                for j in range(Jg):
                    nc.tensor.matmul(
                        acc[:, :nt * FV2], lhsT=ident[:],
                        rhs=rhs2b[:, :SJ * FV2].rearrange(
                            "p (t j f) -> p t (j f)", j=Jg,
                            f=FV2)[:, :, j * FV2:(j + 1) * FV2],
                        start=(j == 0), stop=(j == Jg - 1))

                av = acc[:, :nt * FV2].rearrange("p (t f) -> p t f", f=FV2)
                zr2 = fin.tile([P, NT2], F32, tag="zr2")
                nc.vector.tensor_scalar(
                    out=zr2[:, :nt][:, :, None],
                    in0=av[:, :, 0:1], scalar1=1e-16, scalar2=None, op0=ALU.add)
                nc.vector.reciprocal(zr2[:, :nt], zr2[:, :nt])
                h3f = fin.tile([P, NT2 * C2], F32, tag="h3f")
                h3v = h3f[:, :nt * C2].rearrange("p (t c) -> p t c", c=C2)
                nc.vector.tensor_tensor(
                    out=h3v, in0=av[:, :, 1:],
                    in1=zr2[:, :nt][:, :, None].to_broadcast([P, nt, C2]),
                    op=ALU.mult)
                nc.vector.tensor_tensor(
                    out=h3v, in0=h3v,
                    in1=b2r[:][:, None, :].to_broadcast([P, nt, C2]), op=ALU.add)
                # selu -> h3 bf16
                rr3 = fin.tile([P, NT2 * C2], F32, tag="rr3")
                nc.scalar.activation(rr3[:, :nt * C2], h3f[:, :nt * C2], AF.Relu)
                ww3 = fin.tile([P, NT2 * C2], F32, tag="ww3")
                nc.vector.tensor_tensor(out=ww3[:, :nt * C2], in0=h3f[:, :nt * C2],
                                        in1=rr3[:, :nt * C2], op=ALU.subtract)
                nc.scalar.activation(ww3[:, :nt * C2], ww3[:, :nt * C2], AF.Exp)
                nc.vector.tensor_scalar(out=ww3[:, :nt * C2], in0=ww3[:, :nt * C2],
                                        scalar1=SELU_ALPHA_SCALE,
                                        scalar2=-SELU_ALPHA_SCALE,
                                        op0=ALU.mult, op1=ALU.add)
                nc.vector.tensor_scalar(out=rr3[:, :nt * C2], in0=rr3[:, :nt * C2],
                                        scalar1=SELU_SCALE, scalar2=None,
                                        op0=ALU.mult)
                h3 = fin.tile([P, NT2 * C2], BF16, tag="h3")
                nc.vector.tensor_tensor(out=h3[:, :nt * C2], in0=ww3[:, :nt * C2],
                                        in1=rr3[:, :nt * C2], op=ALU.add)

                # final head per tile
                ost = fin.tile([P, NT2 * NCOUT], F32, tag="ost")
                for i in range(nt):
                    tp = auxp.tile([H1 * FIN, P], BF16, tag="tp")
                    nc.tensor.transpose(tp[:C2, :], h3[:, i * C2:(i + 1) * C2],
                                        ident[:])
                    h3T = pa.tile([C2, P], BF16, tag="h3T")
                    nc.scalar.activation(h3T[:], tp[:C2, :], AF.Copy)
                    pso = psA.tile([P, 8 * SLOTW], F32, tag="ps_big")
                    nc.tensor.matmul(pso[:, :NCOUT], lhsT=h3T[:], rhs=wf[:],
                                     start=True, stop=True)
                    nc.scalar.activation(ost[:, i * NCOUT:(i + 1) * NCOUT],
                                         pso[:, :NCOUT], AF.Copy)
                nc.vector.tensor_tensor(
                    out=ost[:, :nt * NCOUT].rearrange("p (t c) -> p t c", c=NCOUT),
                    in0=ost[:, :nt * NCOUT].rearrange("p (t c) -> p t c", c=NCOUT),
                    in1=bfr[:][:, None, :].to_broadcast([P, nt, NCOUT]), op=ALU.add)
                dst_ap = t_out[t0 * P:(t0 + nt) * P, :].rearrange(
                    "(i p) c -> p i c", p=P)
                nc.sync.dma_start(dst_ap, ost[:, :nt * NCOUT].rearrange(
                    "p (i c) -> p i c", c=NCOUT))

    nc.compile()
    return nc


def _make_inputs(cfg, percore, wpack, inputs):
    b1 = np.asarray(inputs["b1"], np.float32)
    b2 = np.asarray(inputs["b2"], np.float32)
    bf = np.asarray(inputs["bf"], np.float32)
    wf = np.asarray(inputs["Wf"], np.float32).astype(NPBF16)
    b1T = np.ascontiguousarray(b1[:, None])
    b2r = np.broadcast_to(b2, (P, cfg.C2)).copy()
    bfr = np.broadcast_to(bf, (P, cfg.NCOUT)).copy()
    shared = {
        "waug_blk1": wpack["waug_blk1"].astype(NPBF16),
        "wad_blk": wpack["wad_blk"].astype(NPBF16),
        "w1blk": wpack["w1blk"].astype(NPBF16),
        "waug2": wpack["waug2"].astype(NPBF16),
        "a2rep": wpack["a2rep"].astype(NPBF16),
        "wf": wf, "b1T": b1T, "b2r": b2r, "bfr": bfr,
    }
    in_maps = []
    for c in range(cfg.ncores):
        pc = percore[c]
        m = dict(shared)
        m.update({"xstk": pc["xstk"], "xown_stk": pc["xown_stk"],
                  "idx16": pc["idx16"], "m4": pc["m4"], "mask1": pc["mask1"]})
        in_maps.append(m)
    return in_maps


def _assemble(cfg, percore, results):
    out = np.zeros((cfg.NPAD, cfg.NCOUT), np.float32)
    for c in range(cfg.ncores):
        out[percore[c]["own"]] = results[c]["out"]
    return out[:cfg.N]


def kernel(**inputs) -> np.ndarray:
    cfg = Cfg(N=100000, E=800000, ncores=8)
    percore, wpack, meta = preprocess(
        cfg,
        np.asarray(inputs["x"], np.float32),
        np.asarray(inputs["edge_index"]),
        np.asarray(inputs["W1"], np.float32),
        np.asarray(inputs["a_src1"], np.float32),
        np.asarray(inputs["a_dst1"], np.float32),
        np.asarray(inputs["W2"], np.float32),
        np.asarray(inputs["a_src2"], np.float32),
        np.asarray(inputs["a_dst2"], np.float32),
    )
    nc = build_nc(cfg, meta)
    in_maps = _make_inputs(cfg, percore, wpack, inputs)
    res = run_bass_kernel_spmd(nc, in_maps, core_ids=list(range(cfg.ncores)))
    return _assemble(cfg, percore, res.results)


if __name__ == "__main__":
    import reference as R
    inputs = R.setup_inputs()
    out = kernel(**{k: np.asarray(v) for k, v in inputs.items()})
    print("out", out.shape, out.dtype)
